# revision 27
# baseline (speedup 1.0000x reference)
"""Trainium2 Bass kernel for nn_MCC_46076409152266 (dense_transformer).

XCA-style attention block: channels-first LayerNorm -> depthwise/pointwise
convs -> per-head channel-Gram attention over all HW tokens -> softmax(32x32)
-> 1x1 project + residual -> 1x1 FF + lrelu + depthwise 3x3 + lrelu.

Sharding: spatial rows (H) across 8 cores with 2-row reflect halos applied
host-side. All LayerNorm / 1x1 / depthwise ops are core-local; only the tiny
per-head Gram matrices + q/k squared-norms are globally reduced with ONE
on-device AllReduce (~266 KiB).

Host/transfer path (the axon tunnel is ~75-140 MB/s, so bytes moved dominate
wall-clock): the jit(shard_map) executable is compiled ONCE and cached;
weights live on-device (replicated); the donated output operands are the
previous call's output buffers (no zero upload); inputs are two bf16 slabs
(no W padding - reflect columns are built in-kernel); outputs come back in
bf16. Global top/bottom boundary rows (which the reference computes by
reflecting the ff1 activation) are produced on-device as a 2-row side output.
"""

import os
import numpy as np
import ml_dtypes
from concurrent.futures import ThreadPoolExecutor
from contextlib import ExitStack

import jax
import jax.numpy as jnp

import concourse.bass as bass
import concourse.tile as tile
from concourse import bacc, mybir

F32 = mybir.dt.float32
BF16 = mybir.dt.bfloat16
FP8 = mybir.dt.float8e4
OP = mybir.AluOpType
AF = mybir.ActivationFunctionType
AX = mybir.AxisListType
BF = ml_dtypes.bfloat16
E4 = ml_dtypes.float8_e4m3

LN_EPS = 1e-6
NORM_EPS = 1e-12
LRELU_SLOPE = 0.1
P = 128
USE_ACT_LRELU = True

_NCPU = os.cpu_count() or 1
_POOL = ThreadPoolExecutor(min(8, _NCPU)) if _NCPU > 1 else None

# conv-id -> engine ('dve' | 'pe' | 'gp'). conv ids: q0..q{CB-1},
# kv0..kv{2CB-1} (first CB are k, rest are v), ff0..ff{CB-1}
DEFAULT_CONV_ENGINES = {
    "q0": "pe", "q1": "pe",
    "kv0": "pe", "kv1": "pe", "kv2": "pe", "kv3": "pe",
    "ff0": "pe", "ff1": "pe",
}


def _subtiles(total, step=512):
    out = []
    o = 0
    while o < total:
        out.append((o, min(step, total - o)))
        o += out[-1][1]
    return out


def _chunks(lo, hi, step):
    out = []
    r = lo
    while r < hi:
        out.append((r, min(r + step, hi)))
        r = out[-1][1]
    return out


def build_program(B, C, H, W, HEADS, CORES, conv_engines=None, crmax=9,
                  n_devices=None, no_collective=False):
    """Builds the single-core SPMD program. Returns (nc, meta)."""
    if conv_engines is None:
        conv_engines = DEFAULT_CONV_ENGINES
    CB = C // P
    assert C % P == 0
    hd = C // HEADS          # head dim (channels per head)
    HPCB = P // hd           # heads per 128-channel block
    assert hd == 32, "vector.transpose block trick needs 32-ch heads"
    RH = H // CORES
    assert H % CORES == 0
    RIN = RH + 4             # slab rows (xn rows -2 .. RH+2)
    Wp = W + 2               # reflect-padded width (in-SBUF only)
    NCONV = 4 * CB           # q(CB) + kv(2CB) + ff(CB)
    CCN = B * CB * P * P + B * 2 * CB * P

    nc = bacc.Bacc("TRN2", target_bir_lowering=False, debug=False,
                   num_devices=n_devices or CORES)

    # x_KV ships as fp8 e4m3: its quantization noise is averaged away by the
    # global attention (k) and the 32-wide attn@v + 256-wide 1x1 sums (v),
    # verified to cost <2e-4 absmax output error. x_Q must stay bf16 (it
    # feeds the residual path directly).
    xq_d = nc.dram_tensor("xq", [B, C, RIN, W], BF16, kind="ExternalInput").ap()
    xkv_d = nc.dram_tensor("xkv", [B, C, RIN, W], FP8, kind="ExternalInput").ap()
    wkv_d = nc.dram_tensor("wkv_lhsT", [CB, 2 * CB, P, P], BF16, kind="ExternalInput").ap()
    wcat_d = nc.dram_tensor("wcat_lhsT", [2 * CB, CB, P, P], BF16, kind="ExternalInput").ap()
    taps_d = nc.dram_tensor("dwtaps", [P, NCONV, 9], F32, kind="ExternalInput").ap()
    diag_d = nc.dram_tensor("dwdiag", [NCONV, 9, P, P], BF16, kind="ExternalInput").ap()
    tempv_d = nc.dram_tensor("tempv", [P, CB], F32, kind="ExternalInput").ap()
    # rows 0:RH = final output rows; rows RH:RH+2 = boundary candidate rows
    # (top, bottom) for the global image edge fix-up.
    out_d = nc.dram_tensor("out", [B, C, RH + 2, W], BF16,
                           kind="ExternalOutput").ap()

    conv_ids = ([f"q{i}" for i in range(CB)] + [f"kv{i}" for i in range(2 * CB)]
                + [f"ff{i}" for i in range(CB)])
    conv_idx = {n: i for i, n in enumerate(conv_ids)}

    with tile.TileContext(nc) as tc, ExitStack() as ctx:
        # ------- global pools (weights / persistent / psum / dram) -------
        wpool = ctx.enter_context(tc.tile_pool(name="weights", bufs=1))
        dpool = ctx.enter_context(tc.tile_pool(name="dram", bufs=1, space="DRAM"))
        persist = ctx.enter_context(tc.tile_pool(name="persist", bufs=1))
        attn_pool = ctx.enter_context(tc.tile_pool(name="attn", bufs=1))
        scr_pool = ctx.enter_context(tc.tile_pool(name="scratch", bufs=2))

        ps_mean = ctx.enter_context(tc.tile_pool(name="psmean", bufs=1, space="PSUM"))
        ps_m2 = ctx.enter_context(tc.tile_pool(name="psm2", bufs=1, space="PSUM"))
        ps_mm = ctx.enter_context(tc.tile_pool(name="psmm", bufs=2, space="PSUM"))
        ps_gram = ctx.enter_context(tc.tile_pool(name="psgram", bufs=1, space="PSUM"))
        ps_conv = ctx.enter_context(tc.tile_pool(name="psconv", bufs=2, space="PSUM"))

        # ------- resident weights -------
        wkv_sb = {}
        for kc in range(CB):
            for mc in range(2 * CB):
                t = wpool.tile([P, P], BF16, name=f"wkv{kc}_{mc}", tag=f"wkv{kc}_{mc}")
                nc.sync.dma_start(t[:], wkv_d[kc, mc])
                wkv_sb[kc, mc] = t
        wcat_sb = {}
        for kc in range(2 * CB):
            for mc in range(CB):
                t = wpool.tile([P, P], BF16, name=f"wcat{kc}_{mc}", tag=f"wcat{kc}_{mc}")
                nc.sync.dma_start(t[:], wcat_d[kc, mc])
                wcat_sb[kc, mc] = t
        taps_sb = wpool.tile([P, NCONV, 9], F32, name="taps", tag="taps")
        nc.sync.dma_start(taps_sb[:], taps_d[:])
        diag_sb = {}
        for name in conv_ids:
            if conv_engines[name] == "pe":
                for j in range(9):
                    t = wpool.tile([P, P], BF16, name=f"diag{name}_{j}", tag=f"diag{name}_{j}")
                    nc.sync.dma_start(t[:], diag_d[conv_idx[name], j])
                    diag_sb[name, j] = t
        tempv_sb = wpool.tile([P, CB], F32, name="tempv", tag="tempv")
        nc.sync.dma_start(tempv_sb[:], tempv_d[:])
        ones_sb = wpool.tile([P, 1], BF16, name="ones", tag="ones")
        nc.vector.memset(ones_sb[:], 1.0 / C)
        epsln_sb = wpool.tile([P, 1], F32, name="epsln", tag="epsln")
        nc.vector.memset(epsln_sb[:], LN_EPS)

        v_buf = dpool.tile([B, CB, P, RH + 2, W], BF16)
        cc_in = dpool.tile([CCN], F32)
        cc_out = dpool.tile([CCN], F32,
                            addr_space="Shared" if CORES > 4 else "Local")

        evict_flip = [0]

        def lrelu_op(dst_ap, src_ap, from_psum):
            """leaky relu; ACT Lrelu on HW, STT fallback for CoreSim."""
            if USE_ACT_LRELU:
                nc.scalar.activation(dst_ap, src_ap, AF.Prelu, bias=0.0,
                                     scale=1.0, alpha=LRELU_SLOPE)
            elif from_psum:
                tmp = scr_pool.tile([P, 512], F32, name="lrtmp", tag="lrtmp")
                n = src_ap.free_size()
                nc.vector.tensor_copy(tmp[:, :n], src_ap)
                nc.vector.scalar_tensor_tensor(dst_ap, tmp[:, :n], LRELU_SLOPE,
                                               tmp[:, :n], OP.mult, OP.max)
            else:
                nc.vector.scalar_tensor_tensor(dst_ap, src_ap, LRELU_SLOPE,
                                               src_ap, OP.mult, OP.max)

        def evict_copy(dst_ap, src_ap):
            """PSUM -> SBUF copy, alternating DVE / ACT."""
            evict_flip[0] ^= 1
            if evict_flip[0]:
                nc.vector.tensor_copy(dst_ap, src_ap)
            else:
                nc.scalar.copy(dst_ap, src_ap)

        def tap_ap(name, j):
            return taps_sb[:, conv_idx[name], j:j + 1]

        def run_conv(name, win, out_t, crr, fuse_lrelu_to=None):
            """3x3 depthwise valid conv: win [P, crr+2, Wp] -> out [P, crr, W]."""
            eng_name = conv_engines[name]
            if eng_name in ("dve", "gp"):
                eng = nc.vector if eng_name == "dve" else nc.gpsimd
                for j in range(9):
                    ky, kx = divmod(j, 3)
                    in0 = win[:, ky:ky + crr, kx:kx + W]
                    if j == 0:
                        eng.tensor_scalar(out_t[:], in0, tap_ap(name, j), None, OP.mult)
                    else:
                        eng.scalar_tensor_tensor(
                            out_t[:], in0, tap_ap(name, j), out_t[:], OP.mult, OP.add)
                if fuse_lrelu_to is not None:
                    lrelu_op(fuse_lrelu_to[:], out_t[:], from_psum=False)
            else:  # 'pe'
                g = max(1, 512 // W)
                r = 0
                while r < crr:
                    gr = min(g, crr - r)
                    ps = ps_conv.tile([P, 512], F32, name="pec", tag="pec")
                    for j in range(9):
                        ky, kx = divmod(j, 3)
                        rhs = win[:, r + ky:r + ky + gr, kx:kx + W]
                        nc.tensor.matmul(ps[:, :gr * W], diag_sb[name, j], rhs,
                                         start=(j == 0), stop=(j == 8))
                    src = ps[:, :gr * W].rearrange("p (r w) -> p r w", r=gr)
                    if fuse_lrelu_to is not None:
                        lrelu_op(fuse_lrelu_to[:, r:r + gr, :], src, from_psum=True)
                    else:
                        evict_copy(out_t[:, r:r + gr, :], src)
                    r += gr

        # persistent per-batch stat tiles
        gram_sb = {}
        ssq_sb = {}
        for b in range(B):
            for cb in range(CB):
                gram_sb[b, cb] = persist.tile([P, P], F32, name=f"gram{b}_{cb}",
                                              tag=f"gram{b}_{cb}")
            for qk in range(2):
                for cb in range(CB):
                    t = persist.tile([P, 1], F32, name=f"ssq{b}_{qk}_{cb}",
                                     tag=f"ssq{b}_{qk}_{cb}")
                    nc.vector.memset(t[:], 0.0)
                    ssq_sb[b, qk, cb] = t

        # ============ PHASE 1 ============
        p1_chunks = _chunks(-1, RH + 1, crmax)
        with ExitStack() as p1ctx:
            xr_pool = p1ctx.enter_context(tc.tile_pool(name="xraw", bufs=1))
            sq_pool = p1ctx.enter_context(tc.tile_pool(name="xsq", bufs=1))
            st_pool = p1ctx.enter_context(tc.tile_pool(name="stats", bufs=1))
            stb_pool = p1ctx.enter_context(tc.tile_pool(name="statb", bufs=2))
            win_pool = p1ctx.enter_context(tc.tile_pool(name="wins", bufs=1))
            carry_pool = p1ctx.enter_context(tc.tile_pool(name="carry", bufs=1))
            qkv_pool = p1ctx.enter_context(tc.tile_pool(name="qkv", bufs=1))
            tr_pool = p1ctx.enter_context(tc.tile_pool(name="trans", bufs=2))

            # ---- LN stats pre-pass over the whole slab (throughput-friendly;
            # keeps the per-chunk critical path free of the stats chain) ----
            SLABNT = RIN * W
            SK2 = (SLABNT + P - 1) // P
            mean_dr = dpool.tile([B, 2, SK2 * P], BF16)
            inv_dr = dpool.tile([B, 2, SK2 * P], BF16)
            m2_dr = dpool.tile([B, 2, SK2 * P], BF16)
            if SK2 * P > SLABNT:
                padt = st_pool.tile([1, SK2 * P - SLABNT], BF16, name="padt",
                                    tag="padt")
                nc.vector.memset(padt[:], 1.0)
                for b in range(B):
                    for pi in range(2):
                        nc.sync.dma_start(mean_dr[b, pi, SLABNT:].rearrange(
                            "(o n) -> o n", o=1), padt[:])
                        nc.sync.dma_start(m2_dr[b, pi, SLABNT:].rearrange(
                            "(o n) -> o n", o=1), padt[:])
            for b in range(B):
                for pi, x_d in enumerate((xq_d, xkv_d)):
                    for s0, ns in _subtiles(SLABNT):
                        mps = ps_mean.tile([1, 512], F32, name="mps", tag="mps")
                        m2ps = ps_m2.tile([1, 512], F32, name="m2ps", tag="m2ps")
                        for cb in range(CB):
                            src_ap = x_d[b, cb * P:(cb + 1) * P].rearrange(
                                "p r w -> p (r w)")[:, s0:s0 + ns]
                            xr = sq_pool.tile([P, 512], BF16, name=f"ppx{cb}",
                                              tag=f"ppx{cb}", bufs=3)
                            if pi == 0:
                                nc.sync.dma_start(xr[:, :ns], src_ap)
                            else:
                                xr8 = sq_pool.tile([P, 512], FP8,
                                                   name=f"ppx8{cb}",
                                                   tag=f"ppx8{cb}", bufs=3)
                                nc.sync.dma_start(xr8[:, :ns], src_ap)
                                nc.gpsimd.tensor_copy(xr[:, :ns], xr8[:, :ns])
                            xs = sq_pool.tile([P, 512], BF16, name=f"ppsq{cb}",
                                              tag=f"ppsq{cb}", bufs=3)
                            nc.scalar.activation(xs[:, :ns], xr[:, :ns], AF.Square)
                            nc.tensor.matmul(mps[:, :ns], ones_sb[:], xr[:, :ns],
                                             start=(cb == 0), stop=(cb == CB - 1))
                            nc.tensor.matmul(m2ps[:, :ns], ones_sb[:], xs[:, :ns],
                                             start=(cb == 0), stop=(cb == CB - 1))
                        mstg = stb_pool.tile([1, 512], BF16, name="mstg", tag="mstg")
                        m2stg = stb_pool.tile([1, 512], BF16, name="m2stg",
                                              tag="m2stg")
                        evict_copy(mstg[:, :ns], mps[:, :ns])
                        evict_copy(m2stg[:, :ns], m2ps[:, :ns])
                        nc.sync.dma_start(mean_dr[b, pi, s0:s0 + ns].rearrange(
                            "(o n) -> o n", o=1), mstg[:, :ns])
                        nc.sync.dma_start(m2_dr[b, pi, s0:s0 + ns].rearrange(
                            "(o n) -> o n", o=1), m2stg[:, :ns])
                    mean_rs = stb_pool.tile([P, SK2], BF16, name="meanrs",
                                            tag="meanrs")
                    m2_rs = stb_pool.tile([P, SK2], BF16, name="m2rs", tag="m2rs")
                    nc.sync.dma_start(mean_rs[:], mean_dr[b, pi].rearrange(
                        "(p k) -> p k", p=P))
                    nc.sync.dma_start(m2_rs[:], m2_dr[b, pi].rearrange(
                        "(p k) -> p k", p=P))
                    var = stb_pool.tile([P, SK2], F32, name="var", tag="var")
                    nc.vector.scalar_tensor_tensor(var[:], mean_rs[:], 1.0,
                                                   mean_rs[:], OP.mult, OP.mult)
                    nc.vector.tensor_tensor(var[:], m2_rs[:], var[:], OP.subtract)
                    std = stb_pool.tile([P, SK2], F32, name="std", tag="std")
                    nc.scalar.activation(std[:], var[:], AF.Sqrt, bias=epsln_sb[:])
                    rcp = stb_pool.tile([P, SK2], F32, name="rcp", tag="rcp")
                    nc.vector.reciprocal(rcp[:], std[:])
                    ve = stb_pool.tile([P, SK2], F32, name="ve", tag="ve")
                    nc.vector.tensor_scalar(ve[:], var[:], LN_EPS, -0.5,
                                            OP.add, OP.mult)
                    inv_rs = stb_pool.tile([P, SK2], BF16, name="invrs", tag="invrs")
                    nc.vector.scalar_tensor_tensor(ve[:], ve[:], 1.0, rcp[:],
                                                   OP.mult, OP.mult)
                    nc.vector.scalar_tensor_tensor(ve[:], ve[:], 1.0, rcp[:],
                                                   OP.mult, OP.mult)
                    nc.vector.scalar_tensor_tensor(inv_rs[:], ve[:], 1.5, rcp[:],
                                                   OP.add, OP.mult)
                    nc.sync.dma_start(inv_dr[b, pi].rearrange("(p k) -> p k", p=P),
                                      inv_rs[:])

            for b in range(B):
                gram_ps = {cb: ps_gram.tile([P, P], F32, name=f"gps{cb}",
                                            tag=f"gps{cb}") for cb in range(CB)}
                gram_started = {cb: False for cb in range(CB)}
                for ci, (r0, r1) in enumerate(p1_chunks):
                    crr = r1 - r0
                    winr = crr + 2
                    fresh_lo = (r0 - 1) if ci == 0 else (p1_chunks[ci - 1][1] + 1)
                    fresh_hi = r1 + 1
                    fr = fresh_hi - fresh_lo
                    fo = 0 if ci == 0 else 2
                    nt = fr * W

                    xnq_win = [win_pool.tile([P, winr, Wp], BF16, name=f"xnqw{cb}",
                                             tag=f"xnqw{cb}") for cb in range(CB)]
                    kv_win = [win_pool.tile([P, winr, Wp], BF16, name=f"kvw{mc}",
                                            tag=f"kvw{mc}") for mc in range(2 * CB)]
                    if ci > 0:
                        for cb in range(CB):
                            nc.vector.tensor_copy(xnq_win[cb][:, 0:2, :],
                                                  carry_q[cb][:])
                        for mc in range(2 * CB):
                            nc.vector.tensor_copy(kv_win[mc][:, 0:2, :],
                                                  carry_kv[mc][:])

                    xn_dst = {}
                    for pi, (path, x_d) in enumerate((("q", xq_d), ("kv", xkv_d))):
                        xraw = []
                        for cb in range(CB):
                            src_ap = x_d[b, cb * P:(cb + 1) * P,
                                         fresh_lo + 2:fresh_hi + 2, :]
                            xr = xr_pool.tile([P, fr, W], BF16, name=f"xr{path}{cb}",
                                              tag=f"xr{path}{cb}")
                            if path == "q":
                                nc.sync.dma_start(xr[:], src_ap)
                            else:
                                xr8 = xr_pool.tile([P, fr, W], FP8,
                                                   name=f"xr8{cb}", tag=f"xr8{cb}")
                                nc.sync.dma_start(xr8[:], src_ap)
                                nc.gpsimd.tensor_copy(xr[:], xr8[:])
                            xraw.append(xr)
                        t0tok = (fresh_lo + 2) * W
                        # ---- xn = (x - mean) * inv (whole-chunk ops) ----
                        mb = stb_pool.tile([P, nt], BF16, name=f"mb{path}",
                                           tag=f"mb{path}", bufs=1)
                        ib = stb_pool.tile([P, nt], BF16, name=f"ib{path}",
                                           tag=f"ib{path}", bufs=1)
                        nc.sync.dma_start(mb[:], mean_dr[b, pi, t0tok:t0tok + nt]
                                          .rearrange("(o n) -> o n", o=1)
                                          .to_broadcast([P, nt]))
                        nc.sync.dma_start(ib[:], inv_dr[b, pi, t0tok:t0tok + nt]
                                          .rearrange("(o n) -> o n", o=1)
                                          .to_broadcast([P, nt]))
                        for cb in range(CB):
                            xrf = xraw[cb][:].rearrange("p r w -> p (r w)")
                            xc = stb_pool.tile([P, nt], BF16, name=f"xc{path}{cb}",
                                               tag=f"xc{path}{cb}", bufs=1)
                            nc.vector.tensor_tensor(xc[:], xrf, mb[:], OP.subtract)
                            if path == "q":
                                dst = xnq_win[cb][:, fo:fo + fr, 1:W + 1]
                                nc.gpsimd.tensor_tensor(
                                    dst, xc[:].rearrange("p (r w) -> p r w", r=fr),
                                    ib[:].rearrange("p (r w) -> p r w", r=fr),
                                    OP.mult)
                            else:
                                nc.gpsimd.tensor_tensor(xc[:], xc[:], ib[:], OP.mult)
                                xn_dst[cb] = xc

                    # kv 1x1 matmul into kv_win fresh interior (2 rows per psum)
                    r = 0
                    while r < fr:
                        gr = min(max(1, 512 // W), fr - r)
                        ns = gr * W
                        s0 = r * W
                        for mc in range(2 * CB):
                            ps = ps_mm.tile([P, 512], F32, name="mmps", tag="mmps")
                            for kc in range(CB):
                                nc.tensor.matmul(ps[:, :ns], wkv_sb[kc, mc][:],
                                                 xn_dst[kc][:, s0:s0 + ns],
                                                 start=(kc == 0), stop=(kc == CB - 1))
                            evict_copy(kv_win[mc][:, fo + r:fo + r + gr, 1:W + 1],
                                       ps[:, :ns].rearrange("p (r w) -> p r w", r=gr))
                        r += gr

                    # reflect-pad the W columns of the fresh window rows
                    for t in xnq_win + kv_win:
                        nc.vector.tensor_copy(t[:, fo:fo + fr, 0:1],
                                              t[:, fo:fo + fr, 2:3])
                        nc.vector.tensor_copy(t[:, fo:fo + fr, Wp - 1:Wp],
                                              t[:, fo:fo + fr, Wp - 3:Wp - 2])

                    # convs
                    q_t = [qkv_pool.tile([P, crr, W], BF16, name=f"qt{cb}",
                                         tag=f"qt{cb}") for cb in range(CB)]
                    k_t = [qkv_pool.tile([P, crr, W], BF16, name=f"kt{cb}",
                                         tag=f"kt{cb}") for cb in range(CB)]
                    v_t = [qkv_pool.tile([P, crr, W], BF16, name=f"vt{cb}",
                                         tag=f"vt{cb}") for cb in range(CB)]
                    for cb in range(CB):
                        run_conv(f"q{cb}", xnq_win[cb][:], q_t[cb], crr)
                    for mc in range(2 * CB):
                        out_t = k_t[mc] if mc < CB else v_t[mc - CB]
                        run_conv(f"kv{mc}", kv_win[mc][:], out_t, crr)
                    for cb in range(CB):
                        nc.sync.dma_start(v_buf[b, cb, :, r0 + 1:r1 + 1, :], v_t[cb][:])

                    # carry tails for next chunk
                    if ci + 1 < len(p1_chunks):
                        carry_q = [carry_pool.tile([P, 2, Wp], BF16, name=f"cq{cb}",
                                                   tag=f"cq{cb}") for cb in range(CB)]
                        carry_kv = [carry_pool.tile([P, 2, Wp], BF16, name=f"ckv{mc}",
                                                    tag=f"ckv{mc}")
                                    for mc in range(2 * CB)]
                        for cb in range(CB):
                            nc.vector.tensor_copy(carry_q[cb][:],
                                                  xnq_win[cb][:, winr - 2:winr, :])
                        for mc in range(2 * CB):
                            nc.vector.tensor_copy(carry_kv[mc][:],
                                                  kv_win[mc][:, winr - 2:winr, :])

                    # Gram + ssq over owned rows
                    own_lo, own_hi = max(r0, 0), min(r1, RH)
                    if own_hi > own_lo:
                        llo = own_lo - r0
                        ofd = (own_hi - own_lo) * W
                        assert ofd % P == 0
                        for cb in range(CB):
                            for qk, t in ((0, q_t[cb]), (1, k_t[cb])):
                                flat = t[:, llo:llo + (own_hi - own_lo), :].rearrange(
                                    "p r w -> p (r w)")
                                scr = scr_pool.tile([P, ofd], BF16, name="ssqscr",
                                                    tag="ssqscr", bufs=1)
                                part = scr_pool.tile([P, 1], F32, name="ssqpart",
                                                     tag="ssqpart")
                                nc.vector.scalar_tensor_tensor(
                                    scr[:], flat, 1.0, flat, OP.mult, OP.mult,
                                    accum_out=part[:])
                                nc.vector.tensor_tensor(ssq_sb[b, qk, cb][:],
                                                        ssq_sb[b, qk, cb][:],
                                                        part[:], OP.add)
                        ntc = ofd // P
                        last_gram = (ci == len(p1_chunks) - 1)
                        for t128 in range(ntc):
                            for cb in range(CB):
                                qT = tr_pool.tile([P, P], BF16, name=f"qT{cb}",
                                                  tag=f"qT{cb}")
                                kT = tr_pool.tile([P, P], BF16, name=f"kT{cb}",
                                                  tag=f"kT{cb}")
                                qfl = q_t[cb][:, llo:, :].rearrange("p r w -> p (r w)")
                                kfl = k_t[cb][:, llo:, :].rearrange("p r w -> p (r w)")
                                nc.sync.dma_start(qT[:], qfl[:, t128 * P:(t128 + 1) * P],
                                                  transpose=True)
                                nc.sync.dma_start(kT[:], kfl[:, t128 * P:(t128 + 1) * P],
                                                  transpose=True)
                                nc.tensor.matmul(gram_ps[cb][:], qT[:], kT[:],
                                                 start=not gram_started[cb],
                                                 stop=(last_gram and t128 == ntc - 1))
                                gram_started[cb] = True
                for cb in range(CB):
                    nc.vector.tensor_copy(gram_sb[b, cb][:], gram_ps[cb][:])

        # ============ COLLECTIVE ============
        goff = 0
        for b in range(B):
            for cb in range(CB):
                nc.sync.dma_start(
                    cc_in[goff:goff + P * P].rearrange("(p k) -> p k", p=P),
                    gram_sb[b, cb][:])
                goff += P * P
        for b in range(B):
            for qk in range(2):
                for cb in range(CB):
                    nc.sync.dma_start(
                        cc_in[goff:goff + P].rearrange("(p k) -> p k", p=P),
                        ssq_sb[b, qk, cb][:])
                    goff += P
        assert goff == CCN
        if no_collective:
            nc.sync.dma_start(cc_out[:], cc_in[:])
        else:
            nc.gpsimd.collective_compute(
                "AllReduce", OP.add, replica_groups=[list(range(CORES))],
                ins=[cc_in[:].opt()], outs=[cc_out[:].opt()])

        # ============ ATTENTION (tiny, replicated) ============
        lhsT_av = {}
        goff = 0
        gram_r = {}
        ssq_r = {}
        for b in range(B):
            for cb in range(CB):
                t = attn_pool.tile([P, P], F32, name=f"gramr{b}_{cb}",
                                   tag=f"gramr{b}_{cb}")
                nc.sync.dma_start(t[:], cc_out[goff:goff + P * P].rearrange(
                    "(p k) -> p k", p=P))
                gram_r[b, cb] = t
                goff += P * P
        for b in range(B):
            for qk in range(2):
                for cb in range(CB):
                    t = attn_pool.tile([P, 1], F32, name=f"ssqr{b}_{qk}_{cb}",
                                       tag=f"ssqr{b}_{qk}_{cb}")
                    nc.sync.dma_start(t[:], cc_out[goff:goff + P].rearrange(
                        "(p k) -> p k", p=P))
                    ssq_r[b, qk, cb] = t
                    goff += P

        for b in range(B):
            for cb in range(CB):
                facs = []
                for qk in range(2):
                    ssq = ssq_r[b, qk, cb]
                    s = attn_pool.tile([P, 1], F32, name=f"s{b}{qk}{cb}",
                                       tag=f"s{b}{qk}{cb}")
                    nc.scalar.activation(s[:], ssq[:], AF.Sqrt, bias=0.0)
                    r = attn_pool.tile([P, 1], F32, name=f"r{b}{qk}{cb}",
                                       tag=f"r{b}{qk}{cb}")
                    nc.vector.reciprocal(r[:], s[:])
                    s2 = attn_pool.tile([P, 1], F32, name=f"s2{b}{qk}{cb}",
                                        tag=f"s2{b}{qk}{cb}")
                    nc.vector.scalar_tensor_tensor(s2[:], ssq[:], 1.0, r[:],
                                                   OP.mult, OP.mult)
                    nc.vector.tensor_tensor(s2[:], s2[:], s[:], OP.add)
                    nc.vector.tensor_scalar(s2[:], s2[:], 0.5, NORM_EPS,
                                            OP.mult, OP.max)
                    f = attn_pool.tile([P, 1], F32, name=f"f{b}{qk}{cb}",
                                       tag=f"f{b}{qk}{cb}")
                    nc.vector.reciprocal(f[:], s2[:])
                    facs.append(f)
                fq, fk = facs
                fqt = attn_pool.tile([P, 1], F32, name=f"fqt{b}{cb}",
                                     tag=f"fqt{b}{cb}")
                nc.vector.tensor_tensor(fqt[:], fq[:], tempv_sb[:, cb:cb + 1],
                                        OP.mult)
                fkd = dpool.tile([P], F32, name=f"fkd{b}{cb}", tag=f"fkd{b}{cb}")
                nc.sync.dma_start(fkd[:].rearrange("(p k) -> p k", p=P), fk[:])
                fkb = attn_pool.tile([P, P], F32, name=f"fkb{b}{cb}",
                                     tag=f"fkb{b}{cb}")
                nc.sync.dma_start(fkb[:], fkd[:].rearrange("(k p) -> k p", k=1)
                                  .to_broadcast([P, P]))
                lg = attn_pool.tile([P, P], F32, name=f"lg{b}{cb}", tag=f"lg{b}{cb}")
                nc.vector.scalar_tensor_tensor(lg[:], fkb[:], fqt[:],
                                               gram_r[b, cb][:], OP.mult, OP.mult)
                dcp = attn_pool.tile([P, hd], F32, name=f"dcp{b}{cb}",
                                     tag=f"dcp{b}{cb}")
                for i in range(HPCB):
                    nc.vector.tensor_copy(
                        dcp[i * hd:(i + 1) * hd, :],
                        lg[i * hd:(i + 1) * hd, i * hd:(i + 1) * hd])
                rmax = attn_pool.tile([P, 1], F32, name=f"rmax{b}{cb}",
                                      tag=f"rmax{b}{cb}")
                nc.vector.tensor_reduce(rmax[:], dcp[:], AX.X, OP.max)
                nm = attn_pool.tile([P, 1], F32, name=f"nm{b}{cb}", tag=f"nm{b}{cb}")
                nc.vector.tensor_scalar_mul(nm[:], rmax[:], -1.0)
                e = attn_pool.tile([P, hd], F32, name=f"e{b}{cb}", tag=f"e{b}{cb}")
                nc.scalar.activation(e[:], dcp[:], AF.Exp, bias=nm[:])
                rs = attn_pool.tile([P, 1], F32, name=f"rs{b}{cb}", tag=f"rs{b}{cb}")
                nc.vector.tensor_reduce(rs[:], e[:], AX.X, OP.add)
                rr = attn_pool.tile([P, 1], F32, name=f"rr{b}{cb}", tag=f"rr{b}{cb}")
                nc.vector.reciprocal(rr[:], rs[:])
                abf = attn_pool.tile([P, hd], BF16, name=f"abf{b}{cb}",
                                     tag=f"abf{b}{cb}")
                nc.vector.tensor_scalar(abf[:], e[:], rr[:], None, OP.mult)
                aT = attn_pool.tile([P, hd], BF16, name=f"aT{b}{cb}",
                                    tag=f"aT{b}{cb}")
                nc.vector.transpose(aT[:], abf[:])
                lav = attn_pool.tile([P, P], BF16, name=f"lav{b}{cb}",
                                     tag=f"lav{b}{cb}")
                nc.vector.memset(lav[:], 0.0)
                for i in range(HPCB):
                    nc.vector.tensor_copy(
                        lav[i * hd:(i + 1) * hd, i * hd:(i + 1) * hd],
                        aT[i * hd:(i + 1) * hd, :])
                lhsT_av[b, cb] = lav

        # ============ PHASE 2 ============
        p2_chunks = _chunks(0, RH, crmax - 1 if crmax > 2 else crmax)
        with ExitStack() as p2ctx:
            win2_pool = p2ctx.enter_context(tc.tile_pool(name="wins2", bufs=1))
            carry2_pool = p2ctx.enter_context(tc.tile_pool(name="carry2", bufs=1))
            p2_pool = p2ctx.enter_context(tc.tile_pool(name="p2", bufs=1))
            p2b_pool = p2ctx.enter_context(tc.tile_pool(name="p2b", bufs=2))

            for b in range(B):
                for ci, (o0, o1) in enumerate(p2_chunks):
                    cr2 = o1 - o0
                    win2 = cr2 + 2
                    flo = (o0 - 1) if ci == 0 else (p2_chunks[ci - 1][1] + 1)
                    fhi = o1 + 1
                    fr2 = fhi - flo
                    fo = 0 if ci == 0 else 2

                    ff1_win = [win2_pool.tile([P, win2, Wp], BF16, name=f"ff1w{cb}",
                                              tag=f"ff1w{cb}") for cb in range(CB)]
                    if ci > 0:
                        for cb in range(CB):
                            nc.vector.tensor_copy(ff1_win[cb][:, 0:2, :],
                                                  carry_ff[cb][:])

                    vt = []
                    xqt = []
                    for cb in range(CB):
                        v1 = p2_pool.tile([P, fr2, W], BF16, name=f"v2t{cb}",
                                          tag=f"v2t{cb}")
                        nc.sync.dma_start(v1[:], v_buf[b, cb, :, flo + 1:fhi + 1, :])
                        vt.append(v1)
                        x1 = p2_pool.tile([P, fr2, W], BF16, name=f"xq2t{cb}",
                                          tag=f"xq2t{cb}")
                        nc.sync.dma_start(x1[:], xq_d[b, cb * P:(cb + 1) * P,
                                                      flo + 2:fhi + 2, :])
                        xqt.append(x1)

                    grows = max(1, 512 // W)
                    at_sb = [p2_pool.tile([P, fr2, W], BF16, name=f"at{cb}",
                                          tag=f"at{cb}") for cb in range(CB)]
                    r = 0
                    while r < fr2:
                        gr = min(grows, fr2 - r)
                        for cb in range(CB):
                            ps = ps_mm.tile([P, 512], F32, name="mmps", tag="mmps")
                            nc.tensor.matmul(ps[:, :gr * W], lhsT_av[b, cb][:],
                                             vt[cb][:, r:r + gr, :],
                                             start=True, stop=True)
                            evict_copy(at_sb[cb][:, r:r + gr, :],
                                       ps[:, :gr * W].rearrange(
                                           "p (r w) -> p r w", r=gr))
                        for mc in range(CB):
                            ps = ps_conv.tile([P, 512], F32, name="pec", tag="pec")
                            for kc in range(2 * CB):
                                rhs_t = at_sb[kc] if kc < CB else xqt[kc - CB]
                                nc.tensor.matmul(ps[:, :gr * W], wcat_sb[kc, mc][:],
                                                 rhs_t[:, r:r + gr, :],
                                                 start=(kc == 0),
                                                 stop=(kc == 2 * CB - 1))
                            lrelu_op(
                                ff1_win[mc][:, fo + r:fo + r + gr, 1:W + 1],
                                ps[:, :gr * W].rearrange("p (r w) -> p r w", r=gr),
                                from_psum=True)
                        r += gr
                    for cb in range(CB):
                        nc.vector.tensor_copy(ff1_win[cb][:, fo:fo + fr2, 0:1],
                                              ff1_win[cb][:, fo:fo + fr2, 2:3])
                        nc.vector.tensor_copy(ff1_win[cb][:, fo:fo + fr2, Wp - 1:Wp],
                                              ff1_win[cb][:, fo:fo + fr2,
                                                          Wp - 3:Wp - 2])

                    # boundary candidate rows (reference reflects the ff1
                    # activation at the global image edge): top = rows (1,0,1),
                    # bottom = rows (RH-2, RH-1, RH-2). every core computes
                    # both; the host keeps core 0's top and core 7's bottom.
                    bnd_jobs = []
                    if ci == 0:
                        bnd_jobs.append((0, (1, 0, 1)))
                    if ci == len(p2_chunks) - 1:
                        bnd_jobs.append((1, (RH - 2, RH - 1, RH - 2)))
                    for bj, rows in bnd_jobs:
                        for cb in range(CB):
                            bwin = p2b_pool.tile([P, 3, Wp], BF16,
                                                 name=f"bwin{cb}", tag=f"bwin{cb}")
                            for j, rr in enumerate(rows):
                                li = fo + (rr - flo)
                                nc.vector.tensor_copy(bwin[:, j:j + 1, :],
                                                      ff1_win[cb][:, li:li + 1, :])
                            bo = p2b_pool.tile([P, 1, W], BF16, name=f"bo{cb}",
                                               tag=f"bo{cb}")
                            run_conv(f"ff{cb}", bwin[:], bo, 1, fuse_lrelu_to=bo)
                            nc.sync.dma_start(
                                out_d[b, cb * P:(cb + 1) * P,
                                      RH + bj:RH + bj + 1, :], bo[:])

                    if ci + 1 < len(p2_chunks):
                        carry_ff = [carry2_pool.tile([P, 2, Wp], BF16,
                                                     name=f"cff{cb}", tag=f"cff{cb}")
                                    for cb in range(CB)]
                        for cb in range(CB):
                            nc.vector.tensor_copy(carry_ff[cb][:],
                                                  ff1_win[cb][:, win2 - 2:win2, :])
                    for cb in range(CB):
                        co = p2b_pool.tile([P, cr2, W], BF16, name=f"convo{cb}",
                                           tag=f"convo{cb}")
                        fo16 = p2b_pool.tile([P, cr2, W], BF16, name=f"fo16{cb}",
                                             tag=f"fo16{cb}")
                        if conv_engines[f"ff{cb}"] == "pe":
                            run_conv(f"ff{cb}", ff1_win[cb][:], co, cr2,
                                     fuse_lrelu_to=fo16)
                        else:
                            run_conv(f"ff{cb}", ff1_win[cb][:], co, cr2)
                            lrelu_op(fo16[:], co[:], from_psum=False)
                        nc.sync.dma_start(out_d[b, cb * P:(cb + 1) * P, o0:o1, :],
                                          fo16[:])

    nc.compile()
    meta = dict(B=B, C=C, H=H, W=W, HEADS=HEADS, CORES=CORES, RH=RH, RIN=RIN,
                Wp=Wp, CB=CB, NCONV=NCONV, conv_ids=conv_ids)
    return nc, meta


# ---------------------------------------------------------------------------
# host side
# ---------------------------------------------------------------------------

def make_slab(x, meta, bufs=None, dtype=BF):
    """[B,C,H,W] f32 -> [CORES*B, C, RIN, W] slab stack (2-row H halos).

    bufs: optional (xp, G) staging buffers to reuse across calls.
    """
    B, CORES, RH, RIN = meta["B"], meta["CORES"], meta["RH"], meta["RIN"]
    H = x.shape[2]
    if bufs is None:
        xp = np.empty((x.shape[0], x.shape[1], H + 4, x.shape[3]), dtype)
        G = np.empty((CORES * B, x.shape[1], RIN, x.shape[3]), dtype)
    else:
        xp, G = bufs
    x = np.asarray(x)
    C = x.shape[1]
    if _POOL is not None:
        cw = (C + 7) // 8
        list(_POOL.map(lambda j: np.copyto(
            xp[:, j * cw:(j + 1) * cw, 2:H + 2], x[:, j * cw:(j + 1) * cw]),
            range(8)))
    else:
        np.copyto(xp[:, :, 2:H + 2], x)      # f32 -> narrow contiguous cast
    np.copyto(xp[:, :, 0], xp[:, :, 4])      # reflect: row -2 = row 2
    np.copyto(xp[:, :, 1], xp[:, :, 3])      # row -1 = row 1
    np.copyto(xp[:, :, H + 2], xp[:, :, H])  # row H   = row H-2
    np.copyto(xp[:, :, H + 3], xp[:, :, H - 1])
    if _POOL is not None:
        list(_POOL.map(lambda i: np.copyto(
            G[B * i:B * (i + 1)], xp[:, :, RH * i:RH * i + RIN, :]),
            range(CORES)))
    else:
        for i in range(CORES):
            np.copyto(G[B * i:B * (i + 1)], xp[:, :, RH * i:RH * i + RIN, :])
    return G


def make_weights(meta, ln_w, ln_b, temperature, w_q, w_kv_pw, w_kv_dw,
                 w_proj, w_ff1, w_ffdw):
    B, C = meta["B"], meta["C"]
    CB, HEADS, NCONV = meta["CB"], meta["HEADS"], meta["NCONV"]
    hd = C // HEADS
    assert np.allclose(np.asarray(ln_b), 0.0), "nonzero ln_b not supported"
    g = np.asarray(ln_w, np.float32)

    wkv = (np.asarray(w_kv_pw, np.float32) * g[None, :])  # [2C, C]
    wkv_lhsT = np.zeros((CB, 2 * CB, P, P), np.float32)
    for kc in range(CB):
        for mc in range(2 * CB):
            wkv_lhsT[kc, mc] = wkv[mc * P:(mc + 1) * P, kc * P:(kc + 1) * P].T

    w_proj = np.asarray(w_proj, np.float32)
    w_ff1 = np.asarray(w_ff1, np.float32)
    W2 = (w_ff1.astype(np.float64) @ w_proj.astype(np.float64)).astype(np.float32)
    wcat = np.concatenate([W2, w_ff1], axis=1)  # [C, 2C]: out = wcat @ [attn; xq]
    wcat_lhsT = np.zeros((2 * CB, CB, P, P), np.float32)
    for kc in range(2 * CB):
        for mc in range(CB):
            wcat_lhsT[kc, mc] = wcat[mc * P:(mc + 1) * P, kc * P:(kc + 1) * P].T

    wq_t = np.asarray(w_q, np.float32)[:, 0] * g[:, None, None]      # [C,3,3]
    wkvdw_t = np.asarray(w_kv_dw, np.float32)[:, 0]                  # [2C,3,3]
    wff_t = np.asarray(w_ffdw, np.float32)[:, 0]                     # [C,3,3]

    taps = np.zeros((P, NCONV, 9), np.float32)
    diag = np.zeros((NCONV, 9, P, P), np.float32)
    blocks = ([wq_t[i * P:(i + 1) * P] for i in range(CB)]
              + [wkvdw_t[i * P:(i + 1) * P] for i in range(2 * CB)]
              + [wff_t[i * P:(i + 1) * P] for i in range(CB)])
    for ciX, blk in enumerate(blocks):
        for j in range(9):
            ky, kx = divmod(j, 3)
            taps[:, ciX, j] = blk[:, ky, kx]
            np.fill_diagonal(diag[ciX, j], blk[:, ky, kx])

    temp = np.asarray(temperature, np.float32).reshape(HEADS)
    tempv = np.zeros((P, CB), np.float32)
    for cb in range(CB):
        for p in range(P):
            tempv[p, cb] = temp[(cb * P + p) // hd]

    return {
        "wkv_lhsT": wkv_lhsT.astype(BF),
        "wcat_lhsT": wcat_lhsT.astype(BF),
        "dwtaps": taps,
        "dwdiag": diag.astype(BF),
        "tempv": tempv,
    }


class _Runner:
    """Caches the compiled jit(shard_map(bass_exec)) + device-resident weights.

    Per call it only uploads the two input slabs, reuses the previous call's
    (already downloaded) output buffers as the donated output operands, and
    downloads the two bf16 outputs.
    """

    SHARDED_INPUTS = ("xq", "xkv")

    def __init__(self, nc, meta):
        from concourse.bass2jax import (
            install_neuronx_cc_hook, _bass_exec_p, partition_id_tensor,
            fast_dispatch_compile)
        from jax.experimental.shard_map import shard_map
        from jax.sharding import Mesh, PartitionSpec, NamedSharding

        install_neuronx_cc_hook()
        self.meta = meta
        CORES = meta["CORES"]
        devs = jax.devices()[:CORES]
        assert len(devs) == CORES, f"need {CORES} devices, got {len(devs)}"
        mesh = Mesh(np.asarray(devs), ("core",))
        self.sh_core = NamedSharding(mesh, PartitionSpec("core"))
        self.sh_rep = NamedSharding(mesh, PartitionSpec())

        partition_name = (nc.partition_id_tensor.name
                          if nc.partition_id_tensor else None)
        in_names, out_names, out_avals = [], [], []
        for alloc in nc.m.functions[0].allocations:
            if not isinstance(alloc, mybir.MemoryLocationSet):
                continue
            name = alloc.memorylocations[0].name
            if alloc.kind == "ExternalInput":
                if name != partition_name:
                    in_names.append(name)
            elif alloc.kind == "ExternalOutput":
                out_names.append(name)
                out_avals.append(jax.core.ShapedArray(
                    tuple(alloc.tensor_shape), mybir.dt.np(alloc.dtype)))
        self.in_names = in_names
        self.out_names = out_names
        n_params = len(in_names)
        n_outs = len(out_names)
        bind_names = list(in_names) + list(out_names)
        if partition_name is not None:
            bind_names.append(partition_name)

        def _body(*args):
            operands = list(args)
            if partition_name is not None:
                operands.append(partition_id_tensor())
            outs = _bass_exec_p.bind(
                *operands,
                out_avals=tuple(out_avals),
                in_names=tuple(bind_names),
                out_names=tuple(out_names),
                lowering_input_output_aliases=(),
                sim_require_finite=True,
                sim_require_nnan=True,
                nc=nc,
            )
            return tuple(outs)

        # per-core shapes from the BIR; sharded args become global
        # (CORES*d0, ...) with P("core"), weights are replicated P().
        shapes = {}
        for alloc in nc.m.functions[0].allocations:
            if isinstance(alloc, mybir.MemoryLocationSet) and alloc.tensor_shape:
                shapes[alloc.memorylocations[0].name] = (
                    tuple(alloc.tensor_shape), mybir.dt.np(alloc.dtype))

        def is_sharded(name):
            return name in self.SHARDED_INPUTS or name in out_names

        in_specs, sds = [], []
        for name in list(in_names) + list(out_names):
            shp, dt = shapes[name]
            if is_sharded(name):
                in_specs.append(PartitionSpec("core"))
                sds.append(jax.ShapeDtypeStruct((CORES * shp[0], *shp[1:]), dt,
                                                sharding=self.sh_core))
            else:
                in_specs.append(PartitionSpec())
                sds.append(jax.ShapeDtypeStruct(shp, dt, sharding=self.sh_rep))
        out_specs = (PartitionSpec("core"),) * n_outs
        donate = tuple(range(n_params, n_params + n_outs))

        self.compiled = fast_dispatch_compile(
            lambda: jax.jit(
                shard_map(_body, mesh=mesh, in_specs=tuple(in_specs),
                          out_specs=out_specs, check_rep=False),
                donate_argnums=donate, keep_unused=True,
            ).lower(*sds).compile())

        out_g = [((CORES * shapes[n][0][0], *shapes[n][0][1:]), shapes[n][1])
                 for n in out_names]
        self.zeros_maker = jax.jit(
            lambda: tuple(jnp.zeros(s, d) for s, d in out_g),
            out_shardings=(self.sh_core,) * n_outs)

        self.w_np = None
        self.w_dev = None
        self._donate_next = {}
        self._slab_bufs = {}

    def slab_bufs(self, tag, meta, shape, dtype=BF):
        if tag not in self._slab_bufs:
            B, CORES, RIN = meta["B"], meta["CORES"], meta["RIN"]
            xp = np.empty((shape[0], shape[1], shape[2] + 4, shape[3]), dtype)
            G = np.empty((CORES * B, shape[1], RIN, shape[3]), dtype)
            self._slab_bufs[tag] = (xp, G)
        return self._slab_bufs[tag]

    def set_weights(self, w_np):
        if self.w_np is not None and all(
                np.array_equal(self.w_np[k], w_np[k]) for k in w_np):
            return
        self.w_np = w_np
        self.w_dev = {k: jax.device_put(v, self.sh_rep)
                      for k, v in w_np.items()}
        for v in self.w_dev.values():
            v.block_until_ready()

    def put(self, slab):
        return jax.device_put(slab, self.sh_core)

    def launch(self, xq_dev, xkv_dev, slot):
        """Dispatch one (sub-)batch asynchronously; slot indexes the donated
        output-buffer rotation."""
        donate = self._donate_next.get(slot)
        self._donate_next[slot] = None
        if donate is None:
            donate = self.zeros_maker()
        args = []
        for n in self.in_names:
            if n == "xq":
                args.append(xq_dev)
            elif n == "xkv":
                args.append(xkv_dev)
            else:
                args.append(self.w_dev[n])
        outs = self.compiled(*args, *donate)
        outs[0].copy_to_host_async()
        return outs

    def finish(self, outs, u, slot):
        """Per-shard download pipeline: while shard i+1 streams back over the
        tunnel, shard i is upcast (u16<<16 == bf16->f32) and scattered into
        the final [.,C,H,W] f32 (viewed u32) array on the CPU."""
        meta = self.meta
        B, CORES, RH = meta["B"], meta["CORES"], meta["RH"]
        shards = sorted(outs[0].addressable_shards,
                        key=lambda s: s.index[0].start or 0)
        assert len(shards) == CORES

        def one(i):
            g16 = np.asarray(shards[i].data).view(np.uint16)  # [B,C,RH+2,W]
            dst = u[:, :, RH * i:RH * (i + 1), :]
            dst[...] = g16[:, :, :RH, :]
            if i == 0:
                dst[:, :, 0, :] = g16[:, :, RH, :]
            if i == CORES - 1:
                dst[:, :, RH - 1, :] = g16[:, :, RH + 1, :]
            np.left_shift(dst, 16, out=dst)

        if _POOL is not None:
            list(_POOL.map(one, range(CORES)))
        else:
            for i in range(CORES):
                one(i)
        self._donate_next[slot] = outs


_CACHE = {}


def kernel(x_Q, x_KV, ln_w, ln_b, temperature, w_q, w_kv_pw, w_kv_dw,
           w_proj, w_ff1, w_ffdw):
    B, C, H, W = x_Q.shape
    HEADS = int(np.asarray(temperature).shape[0])
    CORES = 8
    key = (C, H, W, HEADS)
    if key not in _CACHE:
        # B=1 program: each batch element is an independent pipelined launch,
        # so the b=1 upload overlaps the b=0 execute/download on the tunnel.
        nc, meta = build_program(1, C, H, W, HEADS, CORES)
        _CACHE[key] = (_Runner(nc, meta), meta)
    runner, meta = _CACHE[key]
    runner.set_weights(make_weights(meta, ln_w, ln_b, temperature, w_q,
                                    w_kv_pw, w_kv_dw, w_proj, w_ff1, w_ffdw))
    x_Q = np.asarray(x_Q)
    x_KV = np.asarray(x_KV)
    u = np.empty((B, C, H, W), np.uint32)
    launches = []
    for b in range(B):
        xq_dev = runner.put(make_slab(
            x_Q[b:b + 1], meta, runner.slab_bufs(("xq", b), meta,
                                                 (1, C, H, W))))
        xkv_dev = runner.put(make_slab(
            x_KV[b:b + 1], meta, runner.slab_bufs(("xkv", b), meta,
                                                  (1, C, H, W), E4),
            dtype=E4))
        launches.append(runner.launch(xq_dev, xkv_dev, slot=b))
    for b in range(B):
        runner.finish(launches[b], u[b:b + 1], slot=b)
    return u.view(np.float32)


# revision 28
# speedup vs baseline: 1.8020x; 1.8020x over previous
"""Trainium2 Bass kernel for nn_MCC_46076409152266 (dense_transformer).

XCA-style attention block: channels-first LayerNorm -> depthwise/pointwise
convs -> per-head channel-Gram attention over all HW tokens -> softmax(32x32)
-> 1x1 project + residual -> 1x1 FF + lrelu + depthwise 3x3 + lrelu.

Sharding: spatial rows (H) across 8 cores with 2-row reflect halos applied
host-side. All LayerNorm / 1x1 / depthwise ops are core-local; only the tiny
per-head Gram matrices + q/k squared-norms are globally reduced with ONE
on-device AllReduce (~266 KiB).

Host/transfer path (the axon tunnel is ~75-140 MB/s, so bytes moved dominate
wall-clock): the jit(shard_map) executable is compiled ONCE and cached;
weights live on-device (replicated); the donated output operands are the
previous call's output buffers (no zero upload); inputs are two bf16 slabs
(no W padding - reflect columns are built in-kernel); outputs come back in
bf16. Global top/bottom boundary rows (which the reference computes by
reflecting the ff1 activation) are produced on-device as a 2-row side output.
"""

import os
import numpy as np
import ml_dtypes
from concurrent.futures import ThreadPoolExecutor
from contextlib import ExitStack

import jax
import jax.numpy as jnp

import concourse.tile as tile
from concourse import bacc, mybir

F32 = mybir.dt.float32
BF16 = mybir.dt.bfloat16
FP8 = mybir.dt.float8e4
OP = mybir.AluOpType
AF = mybir.ActivationFunctionType
AX = mybir.AxisListType
BF = ml_dtypes.bfloat16
E4 = ml_dtypes.float8_e4m3

LN_EPS = 1e-6
NORM_EPS = 1e-12
LRELU_SLOPE = 0.1
P = 128
USE_ACT_LRELU = True

_NCPU = os.cpu_count() or 1
_POOL = ThreadPoolExecutor(min(8, _NCPU)) if _NCPU > 1 else None

# conv-id -> engine ('dve' | 'pe' | 'gp'). conv ids: q0..q{CB-1},
# kv0..kv{2CB-1} (first CB are k, rest are v), ff0..ff{CB-1}
DEFAULT_CONV_ENGINES = {
    "q0": "pe", "q1": "pe",
    "kv0": "pe", "kv1": "pe", "kv2": "pe", "kv3": "pe",
    "ff0": "pe", "ff1": "pe",
}


def _subtiles(total, step=512):
    out = []
    o = 0
    while o < total:
        out.append((o, min(step, total - o)))
        o += out[-1][1]
    return out


def _chunks(lo, hi, step):
    out = []
    r = lo
    while r < hi:
        out.append((r, min(r + step, hi)))
        r = out[-1][1]
    return out


def build_program(B, C, H, W, HEADS, CORES, conv_engines=None, crmax=9,
                  n_devices=None, no_collective=False):
    """Builds the single-core SPMD program. Returns (nc, meta)."""
    if conv_engines is None:
        conv_engines = DEFAULT_CONV_ENGINES
    CB = C // P
    assert C % P == 0
    hd = C // HEADS          # head dim (channels per head)
    HPCB = P // hd           # heads per 128-channel block
    assert hd == 32, "vector.transpose block trick needs 32-ch heads"
    RH = H // CORES
    assert H % CORES == 0
    RIN = RH + 4             # slab rows (xn rows -2 .. RH+2)
    Wp = W + 2               # reflect-padded width (in-SBUF only)
    NCONV = 4 * CB           # q(CB) + kv(2CB) + ff(CB)
    CCN = B * CB * P * P + B * 2 * CB * P

    nc = bacc.Bacc("TRN2", target_bir_lowering=False, debug=False,
                   num_devices=n_devices or CORES)

    # x_KV ships as fp8 e4m3: its quantization noise is averaged away by the
    # global attention (k) and the 32-wide attn@v + 256-wide 1x1 sums (v),
    # verified to cost <2e-4 absmax output error. x_Q must stay bf16 (it
    # feeds the residual path directly).
    xq_d = nc.dram_tensor("xq", [B, C, RIN, W], BF16, kind="ExternalInput").ap()
    xkv_d = nc.dram_tensor("xkv", [B, C, RIN, W], FP8, kind="ExternalInput").ap()
    wkv_d = nc.dram_tensor("wkv_lhsT", [CB, 2 * CB, P, P], BF16, kind="ExternalInput").ap()
    wcat_d = nc.dram_tensor("wcat_lhsT", [2 * CB, CB, P, P], BF16, kind="ExternalInput").ap()
    taps_d = nc.dram_tensor("dwtaps", [P, NCONV, 9], F32, kind="ExternalInput").ap()
    diag_d = nc.dram_tensor("dwdiag", [NCONV, 9, P, P], BF16, kind="ExternalInput").ap()
    tempv_d = nc.dram_tensor("tempv", [P, CB], F32, kind="ExternalInput").ap()
    # rows 0:RH = final output rows; rows RH:RH+2 = boundary candidate rows
    # (top, bottom) for the global image edge fix-up.
    out_d = nc.dram_tensor("out", [B, C, RH + 2, W], BF16,
                           kind="ExternalOutput").ap()

    conv_ids = ([f"q{i}" for i in range(CB)] + [f"kv{i}" for i in range(2 * CB)]
                + [f"ff{i}" for i in range(CB)])
    conv_idx = {n: i for i, n in enumerate(conv_ids)}

    with tile.TileContext(nc) as tc, ExitStack() as ctx:
        # ------- global pools (weights / persistent / psum / dram) -------
        wpool = ctx.enter_context(tc.tile_pool(name="weights", bufs=1))
        dpool = ctx.enter_context(tc.tile_pool(name="dram", bufs=1, space="DRAM"))
        persist = ctx.enter_context(tc.tile_pool(name="persist", bufs=1))
        attn_pool = ctx.enter_context(tc.tile_pool(name="attn", bufs=1))
        scr_pool = ctx.enter_context(tc.tile_pool(name="scratch", bufs=2))

        ps_mean = ctx.enter_context(tc.tile_pool(name="psmean", bufs=1, space="PSUM"))
        ps_m2 = ctx.enter_context(tc.tile_pool(name="psm2", bufs=1, space="PSUM"))
        ps_mm = ctx.enter_context(tc.tile_pool(name="psmm", bufs=2, space="PSUM"))
        ps_gram = ctx.enter_context(tc.tile_pool(name="psgram", bufs=1, space="PSUM"))
        ps_conv = ctx.enter_context(tc.tile_pool(name="psconv", bufs=2, space="PSUM"))

        # ------- resident weights -------
        wkv_sb = {}
        for kc in range(CB):
            for mc in range(2 * CB):
                t = wpool.tile([P, P], BF16, name=f"wkv{kc}_{mc}", tag=f"wkv{kc}_{mc}")
                nc.sync.dma_start(t[:], wkv_d[kc, mc])
                wkv_sb[kc, mc] = t
        wcat_sb = {}
        for kc in range(2 * CB):
            for mc in range(CB):
                t = wpool.tile([P, P], BF16, name=f"wcat{kc}_{mc}", tag=f"wcat{kc}_{mc}")
                nc.sync.dma_start(t[:], wcat_d[kc, mc])
                wcat_sb[kc, mc] = t
        taps_sb = wpool.tile([P, NCONV, 9], F32, name="taps", tag="taps")
        nc.sync.dma_start(taps_sb[:], taps_d[:])
        diag_sb = {}
        for name in conv_ids:
            if conv_engines[name] == "pe":
                for j in range(9):
                    t = wpool.tile([P, P], BF16, name=f"diag{name}_{j}", tag=f"diag{name}_{j}")
                    nc.sync.dma_start(t[:], diag_d[conv_idx[name], j])
                    diag_sb[name, j] = t
        tempv_sb = wpool.tile([P, CB], F32, name="tempv", tag="tempv")
        nc.sync.dma_start(tempv_sb[:], tempv_d[:])
        ones_sb = wpool.tile([P, 1], BF16, name="ones", tag="ones")
        nc.vector.memset(ones_sb[:], 1.0 / C)
        epsln_sb = wpool.tile([P, 1], F32, name="epsln", tag="epsln")
        nc.vector.memset(epsln_sb[:], LN_EPS)

        v_buf = dpool.tile([B, CB, P, RH + 2, W], BF16)
        cc_in = dpool.tile([CCN], F32)
        cc_out = dpool.tile([CCN], F32,
                            addr_space="Shared" if CORES > 4 else "Local")

        evict_flip = [0]

        def lrelu_op(dst_ap, src_ap, from_psum):
            """leaky relu; ACT Lrelu on HW, STT fallback for CoreSim."""
            if USE_ACT_LRELU:
                nc.scalar.activation(dst_ap, src_ap, AF.Prelu, bias=0.0,
                                     scale=1.0, alpha=LRELU_SLOPE)
            elif from_psum:
                tmp = scr_pool.tile([P, 512], F32, name="lrtmp", tag="lrtmp")
                n = src_ap.free_size()
                nc.vector.tensor_copy(tmp[:, :n], src_ap)
                nc.vector.scalar_tensor_tensor(dst_ap, tmp[:, :n], LRELU_SLOPE,
                                               tmp[:, :n], OP.mult, OP.max)
            else:
                nc.vector.scalar_tensor_tensor(dst_ap, src_ap, LRELU_SLOPE,
                                               src_ap, OP.mult, OP.max)

        def evict_copy(dst_ap, src_ap):
            """PSUM -> SBUF copy, alternating DVE / ACT."""
            evict_flip[0] ^= 1
            if evict_flip[0]:
                nc.vector.tensor_copy(dst_ap, src_ap)
            else:
                nc.scalar.copy(dst_ap, src_ap)

        def tap_ap(name, j):
            return taps_sb[:, conv_idx[name], j:j + 1]

        def run_conv(name, win, out_t, crr, fuse_lrelu_to=None):
            """3x3 depthwise valid conv: win [P, crr+2, Wp] -> out [P, crr, W]."""
            eng_name = conv_engines[name]
            if eng_name in ("dve", "gp"):
                eng = nc.vector if eng_name == "dve" else nc.gpsimd
                for j in range(9):
                    ky, kx = divmod(j, 3)
                    in0 = win[:, ky:ky + crr, kx:kx + W]
                    if j == 0:
                        eng.tensor_scalar(out_t[:], in0, tap_ap(name, j), None, OP.mult)
                    else:
                        eng.scalar_tensor_tensor(
                            out_t[:], in0, tap_ap(name, j), out_t[:], OP.mult, OP.add)
                if fuse_lrelu_to is not None:
                    lrelu_op(fuse_lrelu_to[:], out_t[:], from_psum=False)
            else:  # 'pe'
                g = max(1, 512 // W)
                r = 0
                while r < crr:
                    gr = min(g, crr - r)
                    ps = ps_conv.tile([P, 512], F32, name="pec", tag="pec")
                    for j in range(9):
                        ky, kx = divmod(j, 3)
                        rhs = win[:, r + ky:r + ky + gr, kx:kx + W]
                        nc.tensor.matmul(ps[:, :gr * W], diag_sb[name, j], rhs,
                                         start=(j == 0), stop=(j == 8))
                    src = ps[:, :gr * W].rearrange("p (r w) -> p r w", r=gr)
                    if fuse_lrelu_to is not None:
                        lrelu_op(fuse_lrelu_to[:, r:r + gr, :], src, from_psum=True)
                    else:
                        evict_copy(out_t[:, r:r + gr, :], src)
                    r += gr

        # persistent per-batch stat tiles
        gram_sb = {}
        ssq_sb = {}
        for b in range(B):
            for cb in range(CB):
                gram_sb[b, cb] = persist.tile([P, P], F32, name=f"gram{b}_{cb}",
                                              tag=f"gram{b}_{cb}")
            for qk in range(2):
                for cb in range(CB):
                    t = persist.tile([P, 1], F32, name=f"ssq{b}_{qk}_{cb}",
                                     tag=f"ssq{b}_{qk}_{cb}")
                    nc.vector.memset(t[:], 0.0)
                    ssq_sb[b, qk, cb] = t

        # ============ PHASE 1 ============
        p1_chunks = _chunks(-1, RH + 1, crmax)
        with ExitStack() as p1ctx:
            xr_pool = p1ctx.enter_context(tc.tile_pool(name="xraw", bufs=1))
            sq_pool = p1ctx.enter_context(tc.tile_pool(name="xsq", bufs=1))
            st_pool = p1ctx.enter_context(tc.tile_pool(name="stats", bufs=1))
            stb_pool = p1ctx.enter_context(tc.tile_pool(name="statb", bufs=2))
            win_pool = p1ctx.enter_context(tc.tile_pool(name="wins", bufs=1))
            carry_pool = p1ctx.enter_context(tc.tile_pool(name="carry", bufs=1))
            qkv_pool = p1ctx.enter_context(tc.tile_pool(name="qkv", bufs=1))
            tr_pool = p1ctx.enter_context(tc.tile_pool(name="trans", bufs=2))

            # ---- LN stats pre-pass over the whole slab (throughput-friendly;
            # keeps the per-chunk critical path free of the stats chain) ----
            SLABNT = RIN * W
            SK2 = (SLABNT + P - 1) // P
            mean_dr = dpool.tile([B, 2, SK2 * P], BF16)
            inv_dr = dpool.tile([B, 2, SK2 * P], BF16)
            m2_dr = dpool.tile([B, 2, SK2 * P], BF16)
            if SK2 * P > SLABNT:
                padt = st_pool.tile([1, SK2 * P - SLABNT], BF16, name="padt",
                                    tag="padt")
                nc.vector.memset(padt[:], 1.0)
                for b in range(B):
                    for pi in range(2):
                        nc.sync.dma_start(mean_dr[b, pi, SLABNT:].rearrange(
                            "(o n) -> o n", o=1), padt[:])
                        nc.sync.dma_start(m2_dr[b, pi, SLABNT:].rearrange(
                            "(o n) -> o n", o=1), padt[:])
            for b in range(B):
                for pi, x_d in enumerate((xq_d, xkv_d)):
                    for s0, ns in _subtiles(SLABNT):
                        mps = ps_mean.tile([1, 512], F32, name="mps", tag="mps")
                        m2ps = ps_m2.tile([1, 512], F32, name="m2ps", tag="m2ps")
                        for cb in range(CB):
                            src_ap = x_d[b, cb * P:(cb + 1) * P].rearrange(
                                "p r w -> p (r w)")[:, s0:s0 + ns]
                            xr = sq_pool.tile([P, 512], BF16, name=f"ppx{cb}",
                                              tag=f"ppx{cb}", bufs=3)
                            if pi == 0:
                                nc.sync.dma_start(xr[:, :ns], src_ap)
                            else:
                                xr8 = sq_pool.tile([P, 512], FP8,
                                                   name=f"ppx8{cb}",
                                                   tag=f"ppx8{cb}", bufs=3)
                                nc.sync.dma_start(xr8[:, :ns], src_ap)
                                nc.gpsimd.tensor_copy(xr[:, :ns], xr8[:, :ns])
                            xs = sq_pool.tile([P, 512], BF16, name=f"ppsq{cb}",
                                              tag=f"ppsq{cb}", bufs=3)
                            nc.scalar.activation(xs[:, :ns], xr[:, :ns], AF.Square)
                            nc.tensor.matmul(mps[:, :ns], ones_sb[:], xr[:, :ns],
                                             start=(cb == 0), stop=(cb == CB - 1))
                            nc.tensor.matmul(m2ps[:, :ns], ones_sb[:], xs[:, :ns],
                                             start=(cb == 0), stop=(cb == CB - 1))
                        mstg = stb_pool.tile([1, 512], BF16, name="mstg", tag="mstg")
                        m2stg = stb_pool.tile([1, 512], BF16, name="m2stg",
                                              tag="m2stg")
                        evict_copy(mstg[:, :ns], mps[:, :ns])
                        evict_copy(m2stg[:, :ns], m2ps[:, :ns])
                        nc.sync.dma_start(mean_dr[b, pi, s0:s0 + ns].rearrange(
                            "(o n) -> o n", o=1), mstg[:, :ns])
                        nc.sync.dma_start(m2_dr[b, pi, s0:s0 + ns].rearrange(
                            "(o n) -> o n", o=1), m2stg[:, :ns])
                    mean_rs = stb_pool.tile([P, SK2], BF16, name="meanrs",
                                            tag="meanrs")
                    m2_rs = stb_pool.tile([P, SK2], BF16, name="m2rs", tag="m2rs")
                    nc.sync.dma_start(mean_rs[:], mean_dr[b, pi].rearrange(
                        "(p k) -> p k", p=P))
                    nc.sync.dma_start(m2_rs[:], m2_dr[b, pi].rearrange(
                        "(p k) -> p k", p=P))
                    var = stb_pool.tile([P, SK2], F32, name="var", tag="var")
                    nc.vector.scalar_tensor_tensor(var[:], mean_rs[:], 1.0,
                                                   mean_rs[:], OP.mult, OP.mult)
                    nc.vector.tensor_tensor(var[:], m2_rs[:], var[:], OP.subtract)
                    std = stb_pool.tile([P, SK2], F32, name="std", tag="std")
                    nc.scalar.activation(std[:], var[:], AF.Sqrt, bias=epsln_sb[:])
                    rcp = stb_pool.tile([P, SK2], F32, name="rcp", tag="rcp")
                    nc.vector.reciprocal(rcp[:], std[:])
                    ve = stb_pool.tile([P, SK2], F32, name="ve", tag="ve")
                    nc.vector.tensor_scalar(ve[:], var[:], LN_EPS, -0.5,
                                            OP.add, OP.mult)
                    inv_rs = stb_pool.tile([P, SK2], BF16, name="invrs", tag="invrs")
                    nc.vector.scalar_tensor_tensor(ve[:], ve[:], 1.0, rcp[:],
                                                   OP.mult, OP.mult)
                    nc.vector.scalar_tensor_tensor(ve[:], ve[:], 1.0, rcp[:],
                                                   OP.mult, OP.mult)
                    nc.vector.scalar_tensor_tensor(inv_rs[:], ve[:], 1.5, rcp[:],
                                                   OP.add, OP.mult)
                    nc.sync.dma_start(inv_dr[b, pi].rearrange("(p k) -> p k", p=P),
                                      inv_rs[:])

            for b in range(B):
                gram_ps = {cb: ps_gram.tile([P, P], F32, name=f"gps{cb}",
                                            tag=f"gps{cb}") for cb in range(CB)}
                gram_started = {cb: False for cb in range(CB)}
                for ci, (r0, r1) in enumerate(p1_chunks):
                    crr = r1 - r0
                    winr = crr + 2
                    fresh_lo = (r0 - 1) if ci == 0 else (p1_chunks[ci - 1][1] + 1)
                    fresh_hi = r1 + 1
                    fr = fresh_hi - fresh_lo
                    fo = 0 if ci == 0 else 2
                    nt = fr * W

                    xnq_win = [win_pool.tile([P, winr, Wp], BF16, name=f"xnqw{cb}",
                                             tag=f"xnqw{cb}") for cb in range(CB)]
                    kv_win = [win_pool.tile([P, winr, Wp], BF16, name=f"kvw{mc}",
                                            tag=f"kvw{mc}") for mc in range(2 * CB)]
                    if ci > 0:
                        for cb in range(CB):
                            nc.vector.tensor_copy(xnq_win[cb][:, 0:2, :],
                                                  carry_q[cb][:])
                        for mc in range(2 * CB):
                            nc.vector.tensor_copy(kv_win[mc][:, 0:2, :],
                                                  carry_kv[mc][:])

                    xn_dst = {}
                    for pi, (path, x_d) in enumerate((("q", xq_d), ("kv", xkv_d))):
                        xraw = []
                        for cb in range(CB):
                            src_ap = x_d[b, cb * P:(cb + 1) * P,
                                         fresh_lo + 2:fresh_hi + 2, :]
                            xr = xr_pool.tile([P, fr, W], BF16, name=f"xr{path}{cb}",
                                              tag=f"xr{path}{cb}")
                            if path == "q":
                                nc.sync.dma_start(xr[:], src_ap)
                            else:
                                xr8 = xr_pool.tile([P, fr, W], FP8,
                                                   name=f"xr8{cb}", tag=f"xr8{cb}")
                                nc.sync.dma_start(xr8[:], src_ap)
                                nc.gpsimd.tensor_copy(xr[:], xr8[:])
                            xraw.append(xr)
                        t0tok = (fresh_lo + 2) * W
                        # ---- xn = (x - mean) * inv (whole-chunk ops) ----
                        mb = stb_pool.tile([P, nt], BF16, name=f"mb{path}",
                                           tag=f"mb{path}", bufs=1)
                        ib = stb_pool.tile([P, nt], BF16, name=f"ib{path}",
                                           tag=f"ib{path}", bufs=1)
                        nc.sync.dma_start(mb[:], mean_dr[b, pi, t0tok:t0tok + nt]
                                          .rearrange("(o n) -> o n", o=1)
                                          .to_broadcast([P, nt]))
                        nc.sync.dma_start(ib[:], inv_dr[b, pi, t0tok:t0tok + nt]
                                          .rearrange("(o n) -> o n", o=1)
                                          .to_broadcast([P, nt]))
                        for cb in range(CB):
                            xrf = xraw[cb][:].rearrange("p r w -> p (r w)")
                            xc = stb_pool.tile([P, nt], BF16, name=f"xc{path}{cb}",
                                               tag=f"xc{path}{cb}", bufs=1)
                            nc.vector.tensor_tensor(xc[:], xrf, mb[:], OP.subtract)
                            if path == "q":
                                dst = xnq_win[cb][:, fo:fo + fr, 1:W + 1]
                                nc.gpsimd.tensor_tensor(
                                    dst, xc[:].rearrange("p (r w) -> p r w", r=fr),
                                    ib[:].rearrange("p (r w) -> p r w", r=fr),
                                    OP.mult)
                            else:
                                nc.gpsimd.tensor_tensor(xc[:], xc[:], ib[:], OP.mult)
                                xn_dst[cb] = xc

                    # kv 1x1 matmul into kv_win fresh interior (2 rows per psum)
                    r = 0
                    while r < fr:
                        gr = min(max(1, 512 // W), fr - r)
                        ns = gr * W
                        s0 = r * W
                        for mc in range(2 * CB):
                            ps = ps_mm.tile([P, 512], F32, name="mmps", tag="mmps")
                            for kc in range(CB):
                                nc.tensor.matmul(ps[:, :ns], wkv_sb[kc, mc][:],
                                                 xn_dst[kc][:, s0:s0 + ns],
                                                 start=(kc == 0), stop=(kc == CB - 1))
                            evict_copy(kv_win[mc][:, fo + r:fo + r + gr, 1:W + 1],
                                       ps[:, :ns].rearrange("p (r w) -> p r w", r=gr))
                        r += gr

                    # reflect-pad the W columns of the fresh window rows
                    for t in xnq_win + kv_win:
                        nc.vector.tensor_copy(t[:, fo:fo + fr, 0:1],
                                              t[:, fo:fo + fr, 2:3])
                        nc.vector.tensor_copy(t[:, fo:fo + fr, Wp - 1:Wp],
                                              t[:, fo:fo + fr, Wp - 3:Wp - 2])

                    # convs
                    q_t = [qkv_pool.tile([P, crr, W], BF16, name=f"qt{cb}",
                                         tag=f"qt{cb}") for cb in range(CB)]
                    k_t = [qkv_pool.tile([P, crr, W], BF16, name=f"kt{cb}",
                                         tag=f"kt{cb}") for cb in range(CB)]
                    v_t = [qkv_pool.tile([P, crr, W], BF16, name=f"vt{cb}",
                                         tag=f"vt{cb}") for cb in range(CB)]
                    for cb in range(CB):
                        run_conv(f"q{cb}", xnq_win[cb][:], q_t[cb], crr)
                    for mc in range(2 * CB):
                        out_t = k_t[mc] if mc < CB else v_t[mc - CB]
                        run_conv(f"kv{mc}", kv_win[mc][:], out_t, crr)
                    for cb in range(CB):
                        nc.sync.dma_start(v_buf[b, cb, :, r0 + 1:r1 + 1, :], v_t[cb][:])

                    # carry tails for next chunk
                    if ci + 1 < len(p1_chunks):
                        carry_q = [carry_pool.tile([P, 2, Wp], BF16, name=f"cq{cb}",
                                                   tag=f"cq{cb}") for cb in range(CB)]
                        carry_kv = [carry_pool.tile([P, 2, Wp], BF16, name=f"ckv{mc}",
                                                    tag=f"ckv{mc}")
                                    for mc in range(2 * CB)]
                        for cb in range(CB):
                            nc.vector.tensor_copy(carry_q[cb][:],
                                                  xnq_win[cb][:, winr - 2:winr, :])
                        for mc in range(2 * CB):
                            nc.vector.tensor_copy(carry_kv[mc][:],
                                                  kv_win[mc][:, winr - 2:winr, :])

                    # Gram + ssq over owned rows
                    own_lo, own_hi = max(r0, 0), min(r1, RH)
                    if own_hi > own_lo:
                        llo = own_lo - r0
                        ofd = (own_hi - own_lo) * W
                        assert ofd % P == 0
                        for cb in range(CB):
                            for qk, t in ((0, q_t[cb]), (1, k_t[cb])):
                                flat = t[:, llo:llo + (own_hi - own_lo), :].rearrange(
                                    "p r w -> p (r w)")
                                scr = scr_pool.tile([P, ofd], BF16, name="ssqscr",
                                                    tag="ssqscr", bufs=1)
                                part = scr_pool.tile([P, 1], F32, name="ssqpart",
                                                     tag="ssqpart")
                                nc.vector.scalar_tensor_tensor(
                                    scr[:], flat, 1.0, flat, OP.mult, OP.mult,
                                    accum_out=part[:])
                                nc.vector.tensor_tensor(ssq_sb[b, qk, cb][:],
                                                        ssq_sb[b, qk, cb][:],
                                                        part[:], OP.add)
                        ntc = ofd // P
                        last_gram = (ci == len(p1_chunks) - 1)
                        for t128 in range(ntc):
                            for cb in range(CB):
                                qT = tr_pool.tile([P, P], BF16, name=f"qT{cb}",
                                                  tag=f"qT{cb}")
                                kT = tr_pool.tile([P, P], BF16, name=f"kT{cb}",
                                                  tag=f"kT{cb}")
                                qfl = q_t[cb][:, llo:, :].rearrange("p r w -> p (r w)")
                                kfl = k_t[cb][:, llo:, :].rearrange("p r w -> p (r w)")
                                nc.sync.dma_start(qT[:], qfl[:, t128 * P:(t128 + 1) * P],
                                                  transpose=True)
                                nc.sync.dma_start(kT[:], kfl[:, t128 * P:(t128 + 1) * P],
                                                  transpose=True)
                                nc.tensor.matmul(gram_ps[cb][:], qT[:], kT[:],
                                                 start=not gram_started[cb],
                                                 stop=(last_gram and t128 == ntc - 1))
                                gram_started[cb] = True
                for cb in range(CB):
                    nc.vector.tensor_copy(gram_sb[b, cb][:], gram_ps[cb][:])

        # ============ COLLECTIVE ============
        goff = 0
        for b in range(B):
            for cb in range(CB):
                nc.sync.dma_start(
                    cc_in[goff:goff + P * P].rearrange("(p k) -> p k", p=P),
                    gram_sb[b, cb][:])
                goff += P * P
        for b in range(B):
            for qk in range(2):
                for cb in range(CB):
                    nc.sync.dma_start(
                        cc_in[goff:goff + P].rearrange("(p k) -> p k", p=P),
                        ssq_sb[b, qk, cb][:])
                    goff += P
        assert goff == CCN
        if no_collective:
            nc.sync.dma_start(cc_out[:], cc_in[:])
        else:
            nc.gpsimd.collective_compute(
                "AllReduce", OP.add, replica_groups=[list(range(CORES))],
                ins=[cc_in[:].opt()], outs=[cc_out[:].opt()])

        # ============ ATTENTION (tiny, replicated) ============
        lhsT_av = {}
        goff = 0
        gram_r = {}
        ssq_r = {}
        for b in range(B):
            for cb in range(CB):
                t = attn_pool.tile([P, P], F32, name=f"gramr{b}_{cb}",
                                   tag=f"gramr{b}_{cb}")
                nc.sync.dma_start(t[:], cc_out[goff:goff + P * P].rearrange(
                    "(p k) -> p k", p=P))
                gram_r[b, cb] = t
                goff += P * P
        for b in range(B):
            for qk in range(2):
                for cb in range(CB):
                    t = attn_pool.tile([P, 1], F32, name=f"ssqr{b}_{qk}_{cb}",
                                       tag=f"ssqr{b}_{qk}_{cb}")
                    nc.sync.dma_start(t[:], cc_out[goff:goff + P].rearrange(
                        "(p k) -> p k", p=P))
                    ssq_r[b, qk, cb] = t
                    goff += P

        for b in range(B):
            for cb in range(CB):
                facs = []
                for qk in range(2):
                    ssq = ssq_r[b, qk, cb]
                    s = attn_pool.tile([P, 1], F32, name=f"s{b}{qk}{cb}",
                                       tag=f"s{b}{qk}{cb}")
                    nc.scalar.activation(s[:], ssq[:], AF.Sqrt, bias=0.0)
                    r = attn_pool.tile([P, 1], F32, name=f"r{b}{qk}{cb}",
                                       tag=f"r{b}{qk}{cb}")
                    nc.vector.reciprocal(r[:], s[:])
                    s2 = attn_pool.tile([P, 1], F32, name=f"s2{b}{qk}{cb}",
                                        tag=f"s2{b}{qk}{cb}")
                    nc.vector.scalar_tensor_tensor(s2[:], ssq[:], 1.0, r[:],
                                                   OP.mult, OP.mult)
                    nc.vector.tensor_tensor(s2[:], s2[:], s[:], OP.add)
                    nc.vector.tensor_scalar(s2[:], s2[:], 0.5, NORM_EPS,
                                            OP.mult, OP.max)
                    f = attn_pool.tile([P, 1], F32, name=f"f{b}{qk}{cb}",
                                       tag=f"f{b}{qk}{cb}")
                    nc.vector.reciprocal(f[:], s2[:])
                    facs.append(f)
                fq, fk = facs
                fqt = attn_pool.tile([P, 1], F32, name=f"fqt{b}{cb}",
                                     tag=f"fqt{b}{cb}")
                nc.vector.tensor_tensor(fqt[:], fq[:], tempv_sb[:, cb:cb + 1],
                                        OP.mult)
                fkd = dpool.tile([P], F32, name=f"fkd{b}{cb}", tag=f"fkd{b}{cb}")
                nc.sync.dma_start(fkd[:].rearrange("(p k) -> p k", p=P), fk[:])
                fkb = attn_pool.tile([P, P], F32, name=f"fkb{b}{cb}",
                                     tag=f"fkb{b}{cb}")
                nc.sync.dma_start(fkb[:], fkd[:].rearrange("(k p) -> k p", k=1)
                                  .to_broadcast([P, P]))
                lg = attn_pool.tile([P, P], F32, name=f"lg{b}{cb}", tag=f"lg{b}{cb}")
                nc.vector.scalar_tensor_tensor(lg[:], fkb[:], fqt[:],
                                               gram_r[b, cb][:], OP.mult, OP.mult)
                dcp = attn_pool.tile([P, hd], F32, name=f"dcp{b}{cb}",
                                     tag=f"dcp{b}{cb}")
                for i in range(HPCB):
                    nc.vector.tensor_copy(
                        dcp[i * hd:(i + 1) * hd, :],
                        lg[i * hd:(i + 1) * hd, i * hd:(i + 1) * hd])
                rmax = attn_pool.tile([P, 1], F32, name=f"rmax{b}{cb}",
                                      tag=f"rmax{b}{cb}")
                nc.vector.tensor_reduce(rmax[:], dcp[:], AX.X, OP.max)
                nm = attn_pool.tile([P, 1], F32, name=f"nm{b}{cb}", tag=f"nm{b}{cb}")
                nc.vector.tensor_scalar_mul(nm[:], rmax[:], -1.0)
                e = attn_pool.tile([P, hd], F32, name=f"e{b}{cb}", tag=f"e{b}{cb}")
                nc.scalar.activation(e[:], dcp[:], AF.Exp, bias=nm[:])
                rs = attn_pool.tile([P, 1], F32, name=f"rs{b}{cb}", tag=f"rs{b}{cb}")
                nc.vector.tensor_reduce(rs[:], e[:], AX.X, OP.add)
                rr = attn_pool.tile([P, 1], F32, name=f"rr{b}{cb}", tag=f"rr{b}{cb}")
                nc.vector.reciprocal(rr[:], rs[:])
                abf = attn_pool.tile([P, hd], BF16, name=f"abf{b}{cb}",
                                     tag=f"abf{b}{cb}")
                nc.vector.tensor_scalar(abf[:], e[:], rr[:], None, OP.mult)
                aT = attn_pool.tile([P, hd], BF16, name=f"aT{b}{cb}",
                                    tag=f"aT{b}{cb}")
                nc.vector.transpose(aT[:], abf[:])
                lav = attn_pool.tile([P, P], BF16, name=f"lav{b}{cb}",
                                     tag=f"lav{b}{cb}")
                nc.vector.memset(lav[:], 0.0)
                for i in range(HPCB):
                    nc.vector.tensor_copy(
                        lav[i * hd:(i + 1) * hd, i * hd:(i + 1) * hd],
                        aT[i * hd:(i + 1) * hd, :])
                lhsT_av[b, cb] = lav

        # ============ PHASE 2 ============
        p2_chunks = _chunks(0, RH, crmax - 1 if crmax > 2 else crmax)
        with ExitStack() as p2ctx:
            win2_pool = p2ctx.enter_context(tc.tile_pool(name="wins2", bufs=1))
            carry2_pool = p2ctx.enter_context(tc.tile_pool(name="carry2", bufs=1))
            p2_pool = p2ctx.enter_context(tc.tile_pool(name="p2", bufs=1))
            p2b_pool = p2ctx.enter_context(tc.tile_pool(name="p2b", bufs=2))

            for b in range(B):
                for ci, (o0, o1) in enumerate(p2_chunks):
                    cr2 = o1 - o0
                    win2 = cr2 + 2
                    flo = (o0 - 1) if ci == 0 else (p2_chunks[ci - 1][1] + 1)
                    fhi = o1 + 1
                    fr2 = fhi - flo
                    fo = 0 if ci == 0 else 2

                    ff1_win = [win2_pool.tile([P, win2, Wp], BF16, name=f"ff1w{cb}",
                                              tag=f"ff1w{cb}") for cb in range(CB)]
                    if ci > 0:
                        for cb in range(CB):
                            nc.vector.tensor_copy(ff1_win[cb][:, 0:2, :],
                                                  carry_ff[cb][:])

                    vt = []
                    xqt = []
                    for cb in range(CB):
                        v1 = p2_pool.tile([P, fr2, W], BF16, name=f"v2t{cb}",
                                          tag=f"v2t{cb}")
                        nc.sync.dma_start(v1[:], v_buf[b, cb, :, flo + 1:fhi + 1, :])
                        vt.append(v1)
                        x1 = p2_pool.tile([P, fr2, W], BF16, name=f"xq2t{cb}",
                                          tag=f"xq2t{cb}")
                        nc.sync.dma_start(x1[:], xq_d[b, cb * P:(cb + 1) * P,
                                                      flo + 2:fhi + 2, :])
                        xqt.append(x1)

                    grows = max(1, 512 // W)
                    at_sb = [p2_pool.tile([P, fr2, W], BF16, name=f"at{cb}",
                                          tag=f"at{cb}") for cb in range(CB)]
                    r = 0
                    while r < fr2:
                        gr = min(grows, fr2 - r)
                        for cb in range(CB):
                            ps = ps_mm.tile([P, 512], F32, name="mmps", tag="mmps")
                            nc.tensor.matmul(ps[:, :gr * W], lhsT_av[b, cb][:],
                                             vt[cb][:, r:r + gr, :],
                                             start=True, stop=True)
                            evict_copy(at_sb[cb][:, r:r + gr, :],
                                       ps[:, :gr * W].rearrange(
                                           "p (r w) -> p r w", r=gr))
                        for mc in range(CB):
                            ps = ps_conv.tile([P, 512], F32, name="pec", tag="pec")
                            for kc in range(2 * CB):
                                rhs_t = at_sb[kc] if kc < CB else xqt[kc - CB]
                                nc.tensor.matmul(ps[:, :gr * W], wcat_sb[kc, mc][:],
                                                 rhs_t[:, r:r + gr, :],
                                                 start=(kc == 0),
                                                 stop=(kc == 2 * CB - 1))
                            lrelu_op(
                                ff1_win[mc][:, fo + r:fo + r + gr, 1:W + 1],
                                ps[:, :gr * W].rearrange("p (r w) -> p r w", r=gr),
                                from_psum=True)
                        r += gr
                    for cb in range(CB):
                        nc.vector.tensor_copy(ff1_win[cb][:, fo:fo + fr2, 0:1],
                                              ff1_win[cb][:, fo:fo + fr2, 2:3])
                        nc.vector.tensor_copy(ff1_win[cb][:, fo:fo + fr2, Wp - 1:Wp],
                                              ff1_win[cb][:, fo:fo + fr2,
                                                          Wp - 3:Wp - 2])

                    # boundary candidate rows (reference reflects the ff1
                    # activation at the global image edge): top = rows (1,0,1),
                    # bottom = rows (RH-2, RH-1, RH-2). every core computes
                    # both; the host keeps core 0's top and core 7's bottom.
                    bnd_jobs = []
                    if ci == 0:
                        bnd_jobs.append((0, (1, 0, 1)))
                    if ci == len(p2_chunks) - 1:
                        bnd_jobs.append((1, (RH - 2, RH - 1, RH - 2)))
                    for bj, rows in bnd_jobs:
                        for cb in range(CB):
                            bwin = p2b_pool.tile([P, 3, Wp], BF16,
                                                 name=f"bwin{cb}", tag=f"bwin{cb}")
                            for j, rr in enumerate(rows):
                                li = fo + (rr - flo)
                                nc.vector.tensor_copy(bwin[:, j:j + 1, :],
                                                      ff1_win[cb][:, li:li + 1, :])
                            bo = p2b_pool.tile([P, 1, W], BF16, name=f"bo{cb}",
                                               tag=f"bo{cb}")
                            run_conv(f"ff{cb}", bwin[:], bo, 1, fuse_lrelu_to=bo)
                            nc.sync.dma_start(
                                out_d[b, cb * P:(cb + 1) * P,
                                      RH + bj:RH + bj + 1, :], bo[:])

                    if ci + 1 < len(p2_chunks):
                        carry_ff = [carry2_pool.tile([P, 2, Wp], BF16,
                                                     name=f"cff{cb}", tag=f"cff{cb}")
                                    for cb in range(CB)]
                        for cb in range(CB):
                            nc.vector.tensor_copy(carry_ff[cb][:],
                                                  ff1_win[cb][:, win2 - 2:win2, :])
                    for cb in range(CB):
                        co = p2b_pool.tile([P, cr2, W], BF16, name=f"convo{cb}",
                                           tag=f"convo{cb}")
                        fo16 = p2b_pool.tile([P, cr2, W], BF16, name=f"fo16{cb}",
                                             tag=f"fo16{cb}")
                        if conv_engines[f"ff{cb}"] == "pe":
                            run_conv(f"ff{cb}", ff1_win[cb][:], co, cr2,
                                     fuse_lrelu_to=fo16)
                        else:
                            run_conv(f"ff{cb}", ff1_win[cb][:], co, cr2)
                            lrelu_op(fo16[:], co[:], from_psum=False)
                        nc.sync.dma_start(out_d[b, cb * P:(cb + 1) * P, o0:o1, :],
                                          fo16[:])

    nc.compile()
    meta = dict(B=B, C=C, H=H, W=W, HEADS=HEADS, CORES=CORES, RH=RH, RIN=RIN,
                Wp=Wp, CB=CB, NCONV=NCONV, conv_ids=conv_ids)
    return nc, meta


# ---------------------------------------------------------------------------
# host side
# ---------------------------------------------------------------------------

def make_slab(x, meta, bufs=None, dtype=BF):
    """[B,C,H,W] f32 -> [CORES*B, C, RIN, W] slab stack (2-row H halos).

    bufs: optional (xp, G) staging buffers to reuse across calls.
    """
    B, CORES, RH, RIN = meta["B"], meta["CORES"], meta["RH"], meta["RIN"]
    H = x.shape[2]
    if bufs is None:
        xp = np.empty((x.shape[0], x.shape[1], H + 4, x.shape[3]), dtype)
        G = np.empty((CORES * B, x.shape[1], RIN, x.shape[3]), dtype)
    else:
        xp, G = bufs
    x = np.asarray(x)
    C = x.shape[1]
    if _POOL is not None:
        cw = (C + 7) // 8
        list(_POOL.map(lambda j: np.copyto(
            xp[:, j * cw:(j + 1) * cw, 2:H + 2], x[:, j * cw:(j + 1) * cw]),
            range(8)))
    else:
        np.copyto(xp[:, :, 2:H + 2], x)      # f32 -> narrow contiguous cast
    np.copyto(xp[:, :, 0], xp[:, :, 4])      # reflect: row -2 = row 2
    np.copyto(xp[:, :, 1], xp[:, :, 3])      # row -1 = row 1
    np.copyto(xp[:, :, H + 2], xp[:, :, H])  # row H   = row H-2
    np.copyto(xp[:, :, H + 3], xp[:, :, H - 1])
    if _POOL is not None:
        list(_POOL.map(lambda i: np.copyto(
            G[B * i:B * (i + 1)], xp[:, :, RH * i:RH * i + RIN, :]),
            range(CORES)))
    else:
        for i in range(CORES):
            np.copyto(G[B * i:B * (i + 1)], xp[:, :, RH * i:RH * i + RIN, :])
    return G


def make_weights(meta, ln_w, ln_b, temperature, w_q, w_kv_pw, w_kv_dw,
                 w_proj, w_ff1, w_ffdw):
    B, C = meta["B"], meta["C"]
    CB, HEADS, NCONV = meta["CB"], meta["HEADS"], meta["NCONV"]
    hd = C // HEADS
    assert np.allclose(np.asarray(ln_b), 0.0), "nonzero ln_b not supported"
    g = np.asarray(ln_w, np.float32)

    wkv = (np.asarray(w_kv_pw, np.float32) * g[None, :])  # [2C, C]
    wkv_lhsT = np.zeros((CB, 2 * CB, P, P), np.float32)
    for kc in range(CB):
        for mc in range(2 * CB):
            wkv_lhsT[kc, mc] = wkv[mc * P:(mc + 1) * P, kc * P:(kc + 1) * P].T

    w_proj = np.asarray(w_proj, np.float32)
    w_ff1 = np.asarray(w_ff1, np.float32)
    W2 = (w_ff1.astype(np.float64) @ w_proj.astype(np.float64)).astype(np.float32)
    wcat = np.concatenate([W2, w_ff1], axis=1)  # [C, 2C]: out = wcat @ [attn; xq]
    wcat_lhsT = np.zeros((2 * CB, CB, P, P), np.float32)
    for kc in range(2 * CB):
        for mc in range(CB):
            wcat_lhsT[kc, mc] = wcat[mc * P:(mc + 1) * P, kc * P:(kc + 1) * P].T

    wq_t = np.asarray(w_q, np.float32)[:, 0] * g[:, None, None]      # [C,3,3]
    wkvdw_t = np.asarray(w_kv_dw, np.float32)[:, 0]                  # [2C,3,3]
    wff_t = np.asarray(w_ffdw, np.float32)[:, 0]                     # [C,3,3]

    taps = np.zeros((P, NCONV, 9), np.float32)
    diag = np.zeros((NCONV, 9, P, P), np.float32)
    blocks = ([wq_t[i * P:(i + 1) * P] for i in range(CB)]
              + [wkvdw_t[i * P:(i + 1) * P] for i in range(2 * CB)]
              + [wff_t[i * P:(i + 1) * P] for i in range(CB)])
    for ciX, blk in enumerate(blocks):
        for j in range(9):
            ky, kx = divmod(j, 3)
            taps[:, ciX, j] = blk[:, ky, kx]
            np.fill_diagonal(diag[ciX, j], blk[:, ky, kx])

    temp = np.asarray(temperature, np.float32).reshape(HEADS)
    tempv = np.zeros((P, CB), np.float32)
    for cb in range(CB):
        for p in range(P):
            tempv[p, cb] = temp[(cb * P + p) // hd]

    return {
        "wkv_lhsT": wkv_lhsT.astype(BF),
        "wcat_lhsT": wcat_lhsT.astype(BF),
        "dwtaps": taps,
        "dwdiag": diag.astype(BF),
        "tempv": tempv,
    }


class _Runner:
    """Caches the compiled jit(shard_map(bass_exec)) + device-resident weights.

    Per call it only uploads the two input slabs, reuses the previous call's
    (already downloaded) output buffers as the donated output operands, and
    downloads the two bf16 outputs.
    """

    SHARDED_INPUTS = ("xq", "xkv")

    def __init__(self, nc, meta):
        from concourse.bass2jax import (
            install_neuronx_cc_hook, _bass_exec_p, partition_id_tensor,
            fast_dispatch_compile)
        from jax.experimental.shard_map import shard_map
        from jax.sharding import Mesh, PartitionSpec, NamedSharding

        install_neuronx_cc_hook()
        self.meta = meta
        CORES = meta["CORES"]
        devs = jax.devices()[:CORES]
        assert len(devs) == CORES, f"need {CORES} devices, got {len(devs)}"
        mesh = Mesh(np.asarray(devs), ("core",))
        self.sh_core = NamedSharding(mesh, PartitionSpec("core"))
        self.sh_rep = NamedSharding(mesh, PartitionSpec())

        partition_name = (nc.partition_id_tensor.name
                          if nc.partition_id_tensor else None)
        in_names, out_names, out_avals = [], [], []
        for alloc in nc.m.functions[0].allocations:
            if not isinstance(alloc, mybir.MemoryLocationSet):
                continue
            name = alloc.memorylocations[0].name
            if alloc.kind == "ExternalInput":
                if name != partition_name:
                    in_names.append(name)
            elif alloc.kind == "ExternalOutput":
                out_names.append(name)
                out_avals.append(jax.core.ShapedArray(
                    tuple(alloc.tensor_shape), mybir.dt.np(alloc.dtype)))
        self.in_names = in_names
        self.out_names = out_names
        n_params = len(in_names)
        n_outs = len(out_names)
        bind_names = list(in_names) + list(out_names)
        if partition_name is not None:
            bind_names.append(partition_name)

        def _body(*args):
            operands = list(args)
            if partition_name is not None:
                operands.append(partition_id_tensor())
            outs = _bass_exec_p.bind(
                *operands,
                out_avals=tuple(out_avals),
                in_names=tuple(bind_names),
                out_names=tuple(out_names),
                lowering_input_output_aliases=(),
                sim_require_finite=True,
                sim_require_nnan=True,
                nc=nc,
            )
            return tuple(outs)

        # per-core shapes from the BIR; sharded args become global
        # (CORES*d0, ...) with P("core"), weights are replicated P().
        shapes = {}
        for alloc in nc.m.functions[0].allocations:
            if isinstance(alloc, mybir.MemoryLocationSet) and alloc.tensor_shape:
                shapes[alloc.memorylocations[0].name] = (
                    tuple(alloc.tensor_shape), mybir.dt.np(alloc.dtype))

        def is_sharded(name):
            return name in self.SHARDED_INPUTS or name in out_names

        in_specs, sds = [], []
        for name in list(in_names) + list(out_names):
            shp, dt = shapes[name]
            if is_sharded(name):
                in_specs.append(PartitionSpec("core"))
                sds.append(jax.ShapeDtypeStruct((CORES * shp[0], *shp[1:]), dt,
                                                sharding=self.sh_core))
            else:
                in_specs.append(PartitionSpec())
                sds.append(jax.ShapeDtypeStruct(shp, dt, sharding=self.sh_rep))
        out_specs = (PartitionSpec("core"),) * n_outs
        donate = tuple(range(n_params, n_params + n_outs))

        self.compiled = fast_dispatch_compile(
            lambda: jax.jit(
                shard_map(_body, mesh=mesh, in_specs=tuple(in_specs),
                          out_specs=out_specs, check_rep=False),
                donate_argnums=donate, keep_unused=True,
            ).lower(*sds).compile())

        out_g = [((CORES * shapes[n][0][0], *shapes[n][0][1:]), shapes[n][1])
                 for n in out_names]
        self.zeros_maker = jax.jit(
            lambda: tuple(jnp.zeros(s, d) for s, d in out_g),
            out_shardings=(self.sh_core,) * n_outs)

        self.w_np = None
        self.w_dev = None
        self._donate_next = {}
        self._slab_bufs = {}

    def slab_bufs(self, tag, meta, shape, dtype=BF):
        if tag not in self._slab_bufs:
            B, CORES, RIN = meta["B"], meta["CORES"], meta["RIN"]
            xp = np.empty((shape[0], shape[1], shape[2] + 4, shape[3]), dtype)
            G = np.empty((CORES * B, shape[1], RIN, shape[3]), dtype)
            self._slab_bufs[tag] = (xp, G)
        return self._slab_bufs[tag]

    def set_weights(self, w_np):
        if self.w_np is not None and all(
                np.array_equal(self.w_np[k], w_np[k]) for k in w_np):
            return
        self.w_np = w_np
        self.w_dev = {k: jax.device_put(v, self.sh_rep)
                      for k, v in w_np.items()}
        for v in self.w_dev.values():
            v.block_until_ready()

    def put(self, slab):
        return jax.device_put(slab, self.sh_core)

    def launch(self, xq_dev, xkv_dev, slot):
        """Dispatch one (sub-)batch asynchronously; slot indexes the donated
        output-buffer rotation."""
        donate = self._donate_next.get(slot)
        self._donate_next[slot] = None
        if donate is None:
            donate = self.zeros_maker()
        args = []
        for n in self.in_names:
            if n == "xq":
                args.append(xq_dev)
            elif n == "xkv":
                args.append(xkv_dev)
            else:
                args.append(self.w_dev[n])
        outs = self.compiled(*args, *donate)
        outs[0].copy_to_host_async()
        return outs

    def finish(self, outs, u, slot):
        """Per-shard download pipeline: while shard i+1 streams back over the
        tunnel, shard i is upcast (u16<<16 == bf16->f32) and scattered into
        the final [.,C,H,W] f32 (viewed u32) array on the CPU."""
        meta = self.meta
        B, CORES, RH = meta["B"], meta["CORES"], meta["RH"]
        shards = sorted(outs[0].addressable_shards,
                        key=lambda s: s.index[0].start or 0)
        assert len(shards) == CORES

        def one(i):
            g16 = np.asarray(shards[i].data).view(np.uint16)  # [B,C,RH+2,W]
            dst = u[:, :, RH * i:RH * (i + 1), :]
            dst[...] = g16[:, :, :RH, :]
            if i == 0:
                dst[:, :, 0, :] = g16[:, :, RH, :]
            if i == CORES - 1:
                dst[:, :, RH - 1, :] = g16[:, :, RH + 1, :]
            np.left_shift(dst, 16, out=dst)

        if _POOL is not None:
            list(_POOL.map(one, range(CORES)))
        else:
            for i in range(CORES):
                one(i)
        self._donate_next[slot] = outs


_CACHE = {}


def kernel(x_Q, x_KV, ln_w, ln_b, temperature, w_q, w_kv_pw, w_kv_dw,
           w_proj, w_ff1, w_ffdw):
    B, C, H, W = x_Q.shape
    HEADS = int(np.asarray(temperature).shape[0])
    CORES = 8
    key = (C, H, W, HEADS)
    if key not in _CACHE:
        # B=1 program: each batch element is an independent pipelined launch,
        # so the b=1 upload overlaps the b=0 execute/download on the tunnel.
        nc, meta = build_program(1, C, H, W, HEADS, CORES)
        _CACHE[key] = (_Runner(nc, meta), meta)
    runner, meta = _CACHE[key]
    runner.set_weights(make_weights(meta, ln_w, ln_b, temperature, w_q,
                                    w_kv_pw, w_kv_dw, w_proj, w_ff1, w_ffdw))
    x_Q = np.asarray(x_Q)
    x_KV = np.asarray(x_KV)
    u = np.empty((B, C, H, W), np.uint32)
    launches = []
    for b in range(B):
        xq_dev = runner.put(make_slab(
            x_Q[b:b + 1], meta, runner.slab_bufs(("xq", b), meta,
                                                 (1, C, H, W))))
        xkv_dev = runner.put(make_slab(
            x_KV[b:b + 1], meta, runner.slab_bufs(("xkv", b), meta,
                                                  (1, C, H, W), E4),
            dtype=E4))
        launches.append(runner.launch(xq_dev, xkv_dev, slot=b))
    for b in range(B):
        runner.finish(launches[b], u[b:b + 1], slot=b)
    return u.view(np.float32)
